# revision 1
# baseline (speedup 1.0000x reference)
"""Trainium2 Bass kernel for nn_DynamicBlock (sparse-token attention + MLP block).

Contract: kernel(**inputs) takes the FULL unsharded inputs (as produced by
reference.setup_inputs()) and returns the FULL [B, T, D] output.

Sharding: 8 cores = 4 batches x 2 interleaved query-halves. Each core:
 - computes rmsnorm + K/V projections (+rope on K) for its batch over all T,
 - processes its 256 selected queries: Q proj + rope, causal attention over
   all T keys (GQA 16 q-heads / 8 kv-heads), o-proj, MLP, gated update,
 - returns the 256 updated rows; the host scatters them into a copy of
   hidden_states.

Everything on-device runs in a transposed layout ([feature, token]) so no
on-device transposes or gathers are needed; all index handling (gather of
selected rows, causal tile bounds) is host-side preprocessing baked into the
program at build time. rotate_half for rope is a PE matmul with a signed
permutation matrix (DVE cannot move data across partitions).
"""

import sys

sys.path.insert(0, "/opt/trn_rl_repo")

import numpy as np
import ml_dtypes

import concourse.bass as bass
import concourse.tile as tile
from concourse import mybir
from concourse.bass_utils import run_bass_kernel_spmd
from concourse.vector_clock import ScopedClock, VectorClock

BF16 = mybir.dt.bfloat16
F32 = mybir.dt.float32
AF = mybir.ActivationFunctionType
OP = mybir.AluOpType

B, T, D = 4, 2048, 1024
H, KV, HD = 16, 8, 64
DFF = 4096
KSEL = 512
EPS = 1e-6

NQ = 256          # queries per core
ND = D // 128     # 8 d-tiles
NT = T // 128     # 16 key tiles
NKC = KV * HD // 128   # 4 k-output chunks (2 kv heads each)
NQC = H * HD // 128    # 8 q-output chunks (2 q heads each)
NFC = DFF // 128       # 32 ff chunks
NCORES = 8

# q-head layout: q-chunk tile 2c holds heads (4c, 4c+2) on partition halves
# (kv heads 2c / 2c+1), tile 2c+1 holds (4c+1, 4c+3). kv head of q-head h is h//2.
TILE_HEADS = []
for c in range(4):
    TILE_HEADS.append((4 * c, 4 * c + 2))
    TILE_HEADS.append((4 * c + 1, 4 * c + 3))
HEAD_PERM = np.array([h * HD + i for pair in TILE_HEADS for h in pair for i in range(HD)])


# ---------------------------------------------------------------------------
# walrus workarounds: this toolchain encodes at most ONE semaphore wait per
# instruction. Split the tile tail-drain into per-proc drains and move excess
# waits onto NoOps.
# ---------------------------------------------------------------------------

def _patched_drain_and_barrier(self, tick_clock, wait_clock):
    gc = tick_clock.global_clock
    n = len(gc)
    for i in range(n):
        t = gc[i]
        if t > 0:
            vec = [0] * n
            vec[i] = t
            d = self.nc.sync.drain()
            wait_clock.add_sem_waits(d.ins, ScopedClock({None: VectorClock(vec)}))
    self.nc.all_engine_barrier()
    popped = self.nc._tile_sem_poison_stack.pop()
    assert popped is self._sem_poison
    self.nc.clear_and_free_semaphores(list(self.sems.allocated().values()))
    self.nc.all_engine_barrier()


tile.TileContext._drain_and_barrier = _patched_drain_and_barrier

_MAX_WAITS = 1


def _split_excess_waits(nc):
    for f in nc.m.functions:
        for bb in f.blocks:
            new = []
            for inst in bb.instructions:
                si = inst.sync_info
                if si is not None and si.on_wait is not None and len(si.on_wait) > _MAX_WAITS:
                    waits = list(si.on_wait)
                    excess, keep = waits[:-_MAX_WAITS], waits[-_MAX_WAITS:]
                    k = 0
                    while excess:
                        chunk, excess = excess[:_MAX_WAITS], excess[_MAX_WAITS:]
                        new.append(mybir.InstNoOp(
                            name=f"{inst.name}_ws{k}",
                            engine=inst.engine,
                            sync_info=mybir.SyncInfo(on_wait=chunk, on_update=[])))
                        k += 1
                    inst.sync_info = mybir.SyncInfo(
                        on_wait=keep, on_update=list(si.on_update or []))
                new.append(inst)
            bb.instructions = new


# ---------------------------------------------------------------------------
# device program
# ---------------------------------------------------------------------------

def build_program(qlo, qhi, dbg=False):
    """qlo/qhi: per key-tile [NT] compile-time query ranges (uniform across cores).

    For key tile tt only queries [qlo[tt]:NQ) attend any of its keys; queries in
    [qlo[tt]:qhi[tt]) are partially masked, [qhi[tt]:NQ) fully valid.
    """
    nc = bass.Bass(trn_type="TRN2", target_bir_lowering=False, debug=False)

    def inp(name, shape, dt):
        return nc.dram_tensor(name, shape, dt, kind="ExternalInput").ap()

    hiddenT = inp("hiddenT", [ND, 128, T], BF16)
    selresT = inp("selresT", [ND, 128, NQ], F32)
    qwT = inp("qwT", [ND, 128, H * HD], BF16)
    kwT = inp("kwT", [ND, 128, KV * HD], BF16)
    vwT = inp("vwT", [ND, 128, KV * HD], BF16)
    owT = inp("owT", [NQC, 128, D], BF16)
    gw = inp("gw", [NFC, 128, ND, 128], BF16)
    uw = inp("uw", [NFC, 128, ND, 128], BF16)
    dw = inp("dw", [ND, 128, NFC, 128], BF16)
    qb = inp("qb", [128, NQC], F32)
    kb = inp("kb", [128, NKC], F32)
    vb = inp("vb", [128, KV * HD], F32)
    rope_m = inp("rope_m", [128, 128], BF16)  # signed rotate-half permutation
    cos_q = inp("cos_q", [128, NQ], BF16)
    sin_q = inp("sin_q", [128, NQ], BF16)
    cos_k = inp("cos_k", [128, T], BF16)
    sin_k = inp("sin_k", [128, T], BF16)
    posq = inp("posq", [128, NQ], F32)        # query positions bcast over partitions
    tvals = inp("tvals", [128, NT], F32)      # tvals[p, tt] = tt*128 + p
    gmul = inp("gmul", [128, NQ], F32)        # gating scores bcast over partitions
    selg = inp("selg", [ND, 128, NQ], F32)    # selresT * (1 - g)

    updT = nc.dram_tensor("updT", [ND, 128, NQ], F32, kind="ExternalOutput").ap()
    dbg_o = {}
    if dbg:
        for nm, shp, dt_ in [("d_normT", [ND, 128, T], BF16),
                             ("d_kT", [NKC, 128, T], BF16),
                             ("d_vplus", [NT, 128, KV, HD + 2], BF16),
                             ("d_qrT", [NQC, 128, NQ], BF16),
                             ("d_nselT", [ND, 128, NQ], BF16),
                             ("d_ctxT", [NQC, 128, NQ], BF16),
                             ("d_hTt", [ND, 128, NQ], F32),
                             ("d_n2T", [ND, 128, NQ], BF16),
                             ("d_actT", [NFC, 128, NQ], BF16),
                             ("d_rr", [NQC, 2, 1, NQ], F32),
                             ("d_mask", [NT, 128, NQ], BF16)]:
            dbg_o[nm] = nc.dram_tensor(nm, shp, dt_, kind="ExternalOutput").ap()

    with tile.TileContext(nc, pool_alloc_mode="queue") as tc:
        with tc.tile_pool(name="ps", bufs=8, space="PSUM") as ps, \
             tc.tile_pool(name="persist", bufs=1) as pp, \
             tc.tile_pool(name="rows", bufs=2) as rowp:

            # ---- persistent tiles ------------------------------------------------
            selT = pp.tile([128, ND, NQ], F32, name="selT")
            nselT = pp.tile([128, ND, NQ], BF16, name="nselT")
            ctxT = pp.tile([128, NQC, NQ], BF16, name="ctxT")
            hTt = pp.tile([128, ND, NQ], F32, name="hTt")
            n2T = pp.tile([128, ND, NQ], BF16, name="n2T")
            ones_t = pp.tile([128, 1], BF16, name="ones_t")
            nc.vector.memset(ones_t, 1.0)
            eps_t = pp.tile([1, 1], F32, name="eps_t")
            nc.vector.memset(eps_t, EPS)
            ones_all = pp.tile([128, 128], F32, name="ones_all")
            nc.vector.memset(ones_all, 1.0)

            # small consts
            c_qb = pp.tile([128, NQC], F32, name="c_qb")
            c_kb = pp.tile([128, NKC], F32, name="c_kb")
            c_vb = pp.tile([128, KV * HD], F32, name="c_vb")
            c_rm = pp.tile([128, 128], BF16, name="c_rm")
            c_cq = pp.tile([128, NQ], BF16, name="c_cq")
            c_sq = pp.tile([128, NQ], BF16, name="c_sq")
            c_pos = pp.tile([128, NQ], F32, name="c_pos")
            c_tv = pp.tile([128, NT], F32, name="c_tv")
            c_g = pp.tile([128, NQ], F32, name="c_g")
            for dt in range(ND):
                eng = nc.scalar if dt % 2 == 0 else nc.sync
                eng.dma_start(out=selT[:, dt, :], in_=selresT[dt])
            for t_, s_ in [(c_qb, qb), (c_kb, kb), (c_vb, vb), (c_rm, rope_m),
                           (c_cq, cos_q), (c_sq, sin_q), (c_pos, posq),
                           (c_tv, tvals), (c_g, gmul)]:
                nc.scalar.dma_start(out=t_, in_=s_)

            pA_cm = tc.tile_pool(name="pA", bufs=1)
            pA = pA_cm.__enter__()
            c_ck = pA.tile([128, T], BF16, name="c_ck")
            c_sk = pA.tile([128, T], BF16, name="c_sk")
            nc.scalar.dma_start(out=c_ck, in_=cos_k)
            nc.scalar.dma_start(out=c_sk, in_=sin_k)
            normT = pA.tile([128, ND, T], BF16, name="normT")

            # resident weights: data tiles (selT/hiddenT) are DMA'd first in the
            # phase bodies on nc.sync; weights go on other queues so the first
            # compute isn't stuck behind 12MB of weight traffic.
            w_q = pA.tile([128, ND, H * HD], BF16, name="w_q")
            w_k = pA.tile([128, ND, KV * HD], BF16, name="w_k")
            w_v = pA.tile([128, ND, KV * HD], BF16, name="w_v")
            w_o = pA.tile([128, NQC, D], BF16, name="w_o")
            for dt in range(ND):
                nc.gpsimd.dma_start(out=w_q[:, dt, :], in_=qwT[dt])
            for dt in range(ND):
                nc.gpsimd.dma_start(out=w_k[:, dt, :], in_=kwT[dt])
                nc.gpsimd.dma_start(out=w_v[:, dt, :], in_=vwT[dt])
            for hc in range(NQC):
                nc.gpsimd.dma_start(out=w_o[:, hc, :], in_=owT[hc])

            # ======================================================================
            # Phase 3 (emitted first): selected-row rmsnorm + Q proj + rope
            # ======================================================================
            qrT = pA.tile([128, NQC, NQ], BF16, name="qrT")
            with tc.tile_pool(name="ph3", bufs=3) as p3:
                ssq = ps.tile([128, 512], F32, name="ssq", tag="ps")
                for dt in range(ND):
                    sq3 = p3.tile([128, NQ], BF16, name="sq3")
                    nc.vector.tensor_mul(out=sq3, in0=selT[:, dt, :], in1=selT[:, dt, :])
                    nc.tensor.matmul(ssq[0:1, 0:NQ], lhsT=ones_t, rhs=sq3,
                                     start=(dt == 0), stop=(dt == ND - 1))
                srow3 = rowp.tile([1, NQ], F32, name="srow3", tag="row")
                nc.scalar.activation(out=srow3, in_=ssq[0:1, 0:NQ], func=AF.Sqrt,
                                     bias=eps_t[0:1, 0:1], scale=1.0 / D)
                rrow3 = rowp.tile([1, NQ], F32, name="rrow3", tag="row")
                nc.vector.reciprocal(out=rrow3, in_=srow3)
                rbc3 = ps.tile([128, 512], F32, name="rbc3", tag="ps")
                nc.tensor.matmul(rbc3[:, 0:NQ], lhsT=ones_all[0:1, :], rhs=rrow3,
                                 start=True, stop=True)
                rbc3_sb = p3.tile([128, NQ], F32, name="rbc3_sb")
                nc.vector.tensor_copy(out=rbc3_sb, in_=rbc3[:, 0:NQ])
                rbc3_b = bass.AP(tensor=rbc3_sb.tensor, offset=rbc3_sb.offset,
                                 ap=[rbc3_sb.ap[0], [0, ND], rbc3_sb.ap[1]])
                nc.vector.tensor_mul(out=nselT[:, :, :], in0=selT[:, :, :],
                                     in1=rbc3_b)

                for qc in range(NQC):
                    qps = ps.tile([128, 512], F32, name="qps", tag="ps")
                    for dt in range(ND):
                        nc.tensor.matmul(
                            qps[:, 0:NQ], lhsT=w_q[:, dt, qc * 128:(qc + 1) * 128],
                            rhs=nselT[:, dt, :],
                            start=(dt == 0), stop=(dt == ND - 1))
                    qraw = p3.tile([128, NQ], BF16, name="qraw")
                    nc.vector.tensor_scalar(
                        out=qraw, in0=qps[:, 0:NQ], scalar1=c_qb[:, qc:qc + 1],
                        scalar2=None, op0=OP.add)
                    rotq = ps.tile([128, 512], F32, name="rotq", tag="ps")
                    nc.tensor.matmul(rotq[:, 0:NQ], lhsT=c_rm, rhs=qraw,
                                     start=True, stop=True)
                    dst = qrT[:, qc, :]
                    tmpq = p3.tile([128, NQ], BF16, name="tmpq")
                    nc.vector.tensor_mul(out=tmpq, in0=rotq[:, 0:NQ], in1=c_sq)
                    nc.vector.tensor_mul(out=dst, in0=qraw, in1=c_cq)
                    nc.vector.tensor_add(out=dst, in0=dst, in1=tmpq)

            # ======================================================================
            # Phases 1+2 fused, chunk-major: per 512-token chunk compute rmsnorm,
            # then K (+rope) and V for that chunk so PE work stays dense.
            # ======================================================================
            kT = pA.tile([128, NKC, T], BF16, name="kT")
            vplus = pA.tile([128, NT, KV, HD + 2], BF16, name="vplus")
            nc.vector.memset(vplus[:, :, :, 0:1], 1.0)
            nc.vector.memset(vplus[:, :, :, HD + 1:HD + 2], 1.0)

            with tc.tile_pool(name="ph1", bufs=3) as p1, \
                 tc.tile_pool(name="ph2", bufs=3) as p2:
                for ch_ in range(4):
                    for dt in range(ND):
                        eng = nc.sync if dt % 2 == 0 else nc.scalar
                        eng.dma_start(
                            out=normT[:, dt, ch_ * 512:(ch_ + 1) * 512],
                            in_=hiddenT[dt, :, ch_ * 512:(ch_ + 1) * 512])
                rstd1_row = rowp.tile([1, T], F32, name="rstd1_row", tag="row")
                for ch in range(4):
                    cs = slice(ch * 512, (ch + 1) * 512)
                    ssp = ps.tile([128, 512], F32, name="ssp", tag="ps")
                    for dt in range(ND):
                        sq = p1.tile([128, 512], BF16, name="sq")
                        nc.vector.tensor_mul(
                            out=sq, in0=normT[:, dt, cs], in1=normT[:, dt, cs])
                        nc.tensor.matmul(ssp[0:1, :], lhsT=ones_t, rhs=sq,
                                         start=(dt == 0), stop=(dt == ND - 1))
                    srow = rowp.tile([1, 512], F32, name="srow", tag="row")
                    nc.scalar.activation(out=srow, in_=ssp[0:1, :], func=AF.Sqrt,
                                         bias=eps_t[0:1, 0:1], scale=1.0 / D)
                    nc.vector.reciprocal(out=rstd1_row[:, cs], in_=srow)
                    rbc = ps.tile([128, 512], F32, name="rbc", tag="ps")
                    nc.tensor.matmul(rbc, lhsT=ones_all[0:1, :],
                                     rhs=rstd1_row[0:1, cs],
                                     start=True, stop=True)
                    rbc_sb = p1.tile([128, 512], BF16, name="rbc_sb", bufs=2)
                    nc.vector.tensor_copy(out=rbc_sb, in_=rbc)
                    rbc_b = bass.AP(tensor=rbc_sb.tensor, offset=rbc_sb.offset,
                                    ap=[rbc_sb.ap[0], [0, ND], rbc_sb.ap[1]])
                    nc.vector.tensor_mul(out=normT[:, :, cs],
                                         in0=normT[:, :, cs], in1=rbc_b)

                    # K for this chunk
                    for kc in range(NKC):
                        kps = ps.tile([128, 512], F32, name="kps", tag="ps")
                        for dt in range(ND):
                            nc.tensor.matmul(
                                kps, lhsT=w_k[:, dt, kc * 128:(kc + 1) * 128],
                                rhs=normT[:, dt, cs],
                                start=(dt == 0), stop=(dt == ND - 1))
                        kraw = p2.tile([128, 512], BF16, name="kraw")
                        nc.vector.tensor_scalar(
                            out=kraw, in0=kps, scalar1=c_kb[:, kc:kc + 1],
                            scalar2=None, op0=OP.add)
                        rot = ps.tile([128, 512], F32, name="rot", tag="ps")
                        nc.tensor.matmul(rot, lhsT=c_rm, rhs=kraw,
                                         start=True, stop=True)
                        dst = kT[:, kc, cs]
                        tmp = p2.tile([128, 512], BF16, name="tmp")
                        nc.vector.tensor_mul(out=tmp, in0=rot, in1=c_sk[:, cs])
                        nc.vector.tensor_mul(out=dst, in0=kraw, in1=c_ck[:, cs])
                        nc.vector.tensor_add(out=dst, in0=dst, in1=tmp)

                    # V for this chunk's 4 key tiles
                    for tt in range(ch * 4, ch * 4 + 4):
                        vps = ps.tile([128, 512], F32, name="vps", tag="ps")
                        for dt in range(ND):
                            nc.tensor.matmul(
                                vps, lhsT=normT[:, dt, tt * 128:(tt + 1) * 128],
                                rhs=w_v[:, dt, :],
                                start=(dt == 0), stop=(dt == ND - 1))
                        nc.vector.tensor_add(
                            out=vplus[:, tt, :, 1:HD + 1],
                            in0=vps.rearrange("p (h d) -> p h d", h=KV),
                            in1=c_vb.rearrange("p (h d) -> p h d", h=KV))

            if dbg:
                for dt in range(ND):
                    nc.scalar.dma_start(out=dbg_o["d_normT"][dt], in_=normT[:, dt, :])
                for kc in range(NKC):
                    nc.scalar.dma_start(out=dbg_o["d_kT"][kc], in_=kT[:, kc, :])
                for tt in range(NT):
                    nc.scalar.dma_start(out=dbg_o["d_vplus"][tt], in_=vplus[:, tt, :, :])
                for qc in range(NQC):
                    nc.scalar.dma_start(out=dbg_o["d_qrT"][qc], in_=qrT[:, qc, :])
                for dt in range(ND):
                    nc.scalar.dma_start(out=dbg_o["d_nselT"][dt], in_=nselT[:, dt, :])

            # ======================================================================
            # Phase 4: attention. 2 groups x 2 kv-chunks; per key tile: scores
            # (K=64 row-group pairs), exp, partial causal mask, ctx accumulate.
            # ctx psum pair tile packs 2 heads along free dim; all at parts 0:65.
            # ======================================================================
            with tc.tile_pool(name="ph4", bufs=1) as p4:
                live = [t_ for t_ in range(NT) if qlo[t_] < NQ]
                last_tt = max(live)
                cps_all = {}

                def attn_tloop(kc):
                    for ab in range(2):
                        cps_all[(kc, ab)] = ps.tile([128, 512], F32,
                                                    name=f"cps{kc}{ab}", tag="ps")
                    for tt in range(NT):
                        lo = qlo[tt]
                        hi = qhi[tt]
                        if lo >= NQ:
                            continue
                        mask = None
                        if hi > lo:
                            mask = p4.tile([128, 512], BF16, name="mask", bufs=2)
                            for mh in range(2):
                                nc.vector.tensor_scalar(
                                    out=mask[:, mh * NQ + lo:mh * NQ + hi],
                                    in0=c_pos[:, lo:hi],
                                    scalar1=c_tv[:, tt:tt + 1], scalar2=None,
                                    op0=OP.is_ge)
                            if dbg and kc == 0:
                                nc.scalar.dma_start(
                                    out=dbg_o["d_mask"][tt, :, 0:hi - lo],
                                    in_=mask[:, lo:hi])
                        for half in range(2):
                            # one bank holds the same row-group half of both
                            # q-tiles (A, B) -> same-bank PE writes stay
                            # serial; cross-bank halves run concurrently.
                            hs_ = slice(half * 64, (half + 1) * 64)
                            sp = ps.tile([128, 512], F32, name="sp", tag="ps")
                            for ab in range(2):
                                nc.tensor.matmul(
                                    sp[:, ab * NQ + lo:ab * NQ + NQ],
                                    lhsT=kT[hs_, kc, tt * 128:(tt + 1) * 128],
                                    rhs=qrT[hs_, 2 * kc + ab, lo:NQ],
                                    start=(ab == 0), stop=(ab == 1))
                            pt = p4.tile([128, 2, NQ], BF16, name="pt", bufs=6)
                            nc.scalar.activation(
                                out=pt[:, :, lo:NQ],
                                in_=sp.rearrange("p (h q) -> p h q", h=2)[:, :, lo:NQ],
                                func=AF.Exp)
                            if mask is not None:
                                nc.vector.tensor_mul(
                                    out=pt[:, :, lo:hi],
                                    in0=pt[:, :, lo:hi],
                                    in1=mask.rearrange("p (h q) -> p h q", h=2)[:, :, lo:hi])
                            kvh = 2 * kc + half
                            for ab in range(2):
                                cp = cps_all[(kc, ab)]
                                # start/stop once per PSUM BANK (zero region)
                                nc.tensor.matmul(
                                    cp[0:HD + 1, half * NQ + lo:half * NQ + NQ],
                                    lhsT=vplus[:, tt, kvh, 1:HD + 2],
                                    rhs=pt[:, ab, lo:NQ],
                                    start=(tt == live[0] and half == 0),
                                    stop=(tt == last_tt and half == 1))

                def attn_evict(kc):
                    # scale by 1/rowsum; odd halves relocated to partitions
                    # 64:128 via SBUF->SBUF DMA (DVE can't cross partitions)
                    for ab in range(2):
                        cp = cps_all[(kc, ab)]
                        rr = p4.tile([128, 512], F32, name="rr", bufs=2)
                        nc.vector.reciprocal(out=rr[64:65, :],
                                             in_=cp[HD:HD + 1, :])
                        if dbg:
                            for mh in range(2):
                                nc.scalar.dma_start(
                                    out=dbg_o["d_rr"][2 * kc + ab, mh],
                                    in_=rr[64:65, mh * NQ:(mh + 1) * NQ])
                        rb = ps.tile([128, 512], F32, name="rb", tag="ps")
                        nc.tensor.matmul(rb[0:64, :],
                                         lhsT=ones_all[64:65, 0:64],
                                         rhs=rr[64:65, :],
                                         start=True, stop=True)
                        rb_sb = p4.tile([64, 512], F32, name="rb_sb", bufs=2)
                        nc.vector.tensor_copy(out=rb_sb, in_=rb[0:64, :])
                        nc.vector.tensor_mul(
                            out=ctxT[0:64, 2 * kc + ab, :],
                            in0=cp[0:HD, 0:NQ], in1=rb_sb[:, 0:NQ])
                        stage = p4.tile([64, NQ], BF16, name="stage", bufs=2)
                        nc.vector.tensor_mul(
                            out=stage, in0=cp[0:HD, NQ:2 * NQ],
                            in1=rb_sb[:, NQ:2 * NQ])
                        nc.sync.dma_start(
                            out=ctxT[64:128, 2 * kc + ab, :], in_=stage)

                # software-pipelined: next group's t-loop is emitted before the
                # previous group's eviction so PE never waits on evictions.
                attn_tloop(0)
                attn_tloop(1)
                attn_evict(0)
                attn_tloop(2)
                attn_evict(1)
                attn_tloop(3)
                attn_evict(2)
                attn_evict(3)

            # ======================================================================
            # Phase 5: o-proj + residual -> hTt (fp32)
            # ======================================================================
            for dc in range(ND):
                ops_ = ps.tile([128, 512], F32, name="ops_", tag="ps")
                for hc in range(NQC):
                    nc.tensor.matmul(
                        ops_[:, 0:NQ], lhsT=w_o[:, hc, dc * 128:(dc + 1) * 128],
                        rhs=ctxT[:, hc, :], start=(hc == 0), stop=(hc == NQC - 1))
                nc.vector.tensor_add(out=hTt[:, dc, :], in0=ops_[:, 0:NQ],
                                     in1=selT[:, dc, :])

            if dbg:
                for qc in range(NQC):
                    nc.scalar.dma_start(out=dbg_o["d_ctxT"][qc], in_=ctxT[:, qc, :])


            if dbg:
                for dt in range(ND):
                    nc.scalar.dma_start(out=dbg_o["d_hTt"][dt], in_=hTt[:, dt, :])
            pA_cm.__exit__(None, None, None)

            # ======================================================================
            # Phase 6: rmsnorm2 -> n2T (bf16)
            # ======================================================================
            with tc.tile_pool(name="ph6", bufs=3) as p6:
                ss2 = ps.tile([128, 512], F32, name="ss2", tag="ps")
                for dt in range(ND):
                    sq6 = p6.tile([128, NQ], BF16, name="sq6")
                    nc.vector.tensor_mul(out=sq6, in0=hTt[:, dt, :], in1=hTt[:, dt, :])
                    nc.tensor.matmul(ss2[0:1, 0:NQ], lhsT=ones_t, rhs=sq6,
                                     start=(dt == 0), stop=(dt == ND - 1))
                srow6 = rowp.tile([1, NQ], F32, name="srow6", tag="row")
                nc.scalar.activation(out=srow6, in_=ss2[0:1, 0:NQ], func=AF.Sqrt,
                                     bias=eps_t[0:1, 0:1], scale=1.0 / D)
                rrow6 = rowp.tile([1, NQ], F32, name="rrow6", tag="row")
                nc.vector.reciprocal(out=rrow6, in_=srow6)
                rbc6 = ps.tile([128, 512], F32, name="rbc6", tag="ps")
                nc.tensor.matmul(rbc6[:, 0:NQ], lhsT=ones_all[0:1, :], rhs=rrow6,
                                 start=True, stop=True)
                rbc6_sb = p6.tile([128, NQ], F32, name="rbc6_sb")
                nc.vector.tensor_copy(out=rbc6_sb, in_=rbc6[:, 0:NQ])
                rbc6_b = bass.AP(tensor=rbc6_sb.tensor, offset=rbc6_sb.offset,
                                 ap=[rbc6_sb.ap[0], [0, ND], rbc6_sb.ap[1]])
                nc.vector.tensor_mul(out=n2T[:, :, :], in0=hTt[:, :, :],
                                     in1=rbc6_b)

            if dbg:
                for dt in range(ND):
                    nc.scalar.dma_start(out=dbg_o["d_n2T"][dt], in_=n2T[:, dt, :])

            # ======================================================================
            # Phase 7: MLP (gate/up silu-mul, down) + gated residual update
            # ======================================================================
            actT = pp.tile([128, NFC, NQ], BF16, name="actT")
            with tc.tile_pool(name="ph7w", bufs=10) as p7w, \
                 tc.tile_pool(name="ph7", bufs=3) as p7:
                for fc in range(NFC):
                    wg_t = p7w.tile([128, ND, 128], BF16, name="wg_t")
                    nc.sync.dma_start(out=wg_t, in_=gw[fc])
                    wu_t = p7w.tile([128, ND, 128], BF16, name="wu_t")
                    nc.scalar.dma_start(out=wu_t, in_=uw[fc])
                    gps = ps.tile([128, 512], F32, name="gps", tag="ps")
                    ups = ps.tile([128, 512], F32, name="ups", tag="ps")
                    for dt in range(ND):
                        nc.tensor.matmul(gps[:, 0:NQ], lhsT=wg_t[:, dt, :],
                                         rhs=n2T[:, dt, :],
                                         start=(dt == 0), stop=(dt == ND - 1))
                    for dt in range(ND):
                        nc.tensor.matmul(ups[:, 0:NQ], lhsT=wu_t[:, dt, :],
                                         rhs=n2T[:, dt, :],
                                         start=(dt == 0), stop=(dt == ND - 1))
                    sg = p7.tile([128, NQ], BF16, name="sg")
                    nc.scalar.activation(out=sg, in_=gps[:, 0:NQ], func=AF.Silu)
                    nc.vector.tensor_mul(out=actT[:, fc, :], in0=ups[:, 0:NQ],
                                         in1=sg)

            if dbg:
                for fc in range(NFC):
                    nc.scalar.dma_start(out=dbg_o["d_actT"][fc], in_=actT[:, fc, :])

            with tc.tile_pool(name="ph8w", bufs=3) as p8w, \
                 tc.tile_pool(name="ph8", bufs=3) as p8:
                selgT = p8w.tile([128, ND, NQ], F32, name="selgT", bufs=1)
                for dt in range(ND):
                    nc.sync.dma_start(out=selgT[:, dt, :], in_=selg[dt])
                for dc in range(ND):
                    wd_t = p8w.tile([128, NFC, 128], BF16, name="wd_t")
                    (nc.sync if dc % 2 == 0 else nc.scalar).dma_start(
                        out=wd_t, in_=dw[dc])
                    mps = ps.tile([128, 512], F32, name="mps", tag="ps")
                    for ft in range(NFC):
                        nc.tensor.matmul(mps[:, 0:NQ], lhsT=wd_t[:, ft, :],
                                         rhs=actT[:, ft, :],
                                         start=(ft == 0), stop=(ft == NFC - 1))
                    # updated = selres*(1-g) + g*(h + mlp)
                    f1 = p8.tile([128, NQ], F32, name="f1")
                    nc.vector.tensor_add(out=f1, in0=mps[:, 0:NQ], in1=hTt[:, dc, :])
                    nc.vector.tensor_mul(out=f1, in0=f1, in1=c_g)
                    nc.vector.tensor_add(out=f1, in0=f1, in1=selgT[:, dc, :])
                    nc.gpsimd.dma_start(out=updT[dc], in_=f1)

    _split_excess_waits(nc)
    return nc


# ---------------------------------------------------------------------------
# host side
# ---------------------------------------------------------------------------

def _bf16(x):
    return np.asarray(x, dtype=np.float32).astype(ml_dtypes.bfloat16)


def _rope_matrix():
    """R[k, p] = sign(p) * 1[k == swap(p)]; (R.T @ x)[p] = sign(p)*x[swap(p)].

    rot(x)[p%64 < 32] = -x[p+32], else +x[p-32]  (two stacked 64-dim heads).
    """
    R = np.zeros((128, 128), np.float32)
    for p in range(128):
        base = (p // 64) * 64
        off = p % 64
        if off < 32:
            R[base + off + 32, p] = -1.0
        else:
            R[base + off - 32, p] = 1.0
    return R




def _install_ntff_hook():
    """Shim antenv.axon_hooks (absent in this image) so trace=True works."""
    import types
    try:
        import antenv.axon_hooks  # noqa: F401
        return
    except ImportError:
        pass
    try:
        from trn_agent_boot.trn_boot import _ntff_profile_via_ctypes
        hook = _ntff_profile_via_ctypes("/opt/axon/libaxon_pjrt.so")
    except Exception:
        hook = None
    mod = types.ModuleType("antenv.axon_hooks")
    mod._hook = hook
    mod.set_axon_ntff_profile_hook = lambda h: setattr(mod, "_hook", h)
    mod.get_axon_ntff_profile_hook = lambda: mod._hook
    sys.modules["antenv.axon_hooks"] = mod


def kernel(hidden_states, token_indices, batch_indices, gating_scores, cos, sin,
           ln1_w, ln2_w, q_w, q_b, k_w, k_b, v_w, v_b, o_w, gate_w, up_w, down_w,
           _profile=False, _dbg=False):
    hidden_states = np.asarray(hidden_states, dtype=np.float32)
    token_indices = np.asarray(token_indices).astype(np.int64)
    gating_scores = np.asarray(gating_scores, dtype=np.float32)
    cos = np.asarray(cos, dtype=np.float32)
    sin = np.asarray(sin, dtype=np.float32)
    ln1_w = np.asarray(ln1_w, dtype=np.float32)
    ln2_w = np.asarray(ln2_w, dtype=np.float32)

    topk = token_indices.reshape(B, KSEL)
    gsc = gating_scores.reshape(B, KSEL)

    core_pos = []
    for c in range(NCORES):
        b = c // 2
        core_pos.append(np.asarray(topk[b, c % 2::2], dtype=np.int64))

    qlo = [min(int(np.searchsorted(core_pos[c], tt * 128)) for c in range(NCORES))
           for tt in range(NT)]
    qhi = [max(int(np.searchsorted(core_pos[c], tt * 128 + 126, side="right"))
               for c in range(NCORES))
           for tt in range(NT)]

    nc = build_program(qlo, qhi, dbg=_dbg)

    # ---- weights (shared across cores) ----
    q_w_eff = (np.asarray(q_w, np.float32) * ln1_w[None, :]) / 8.0
    k_w_eff = np.asarray(k_w, np.float32) * ln1_w[None, :]
    v_w_eff = np.asarray(v_w, np.float32) * ln1_w[None, :]
    g_w_eff = np.asarray(gate_w, np.float32) * ln2_w[None, :]
    u_w_eff = np.asarray(up_w, np.float32) * ln2_w[None, :]
    q_b_eff = (np.asarray(q_b, np.float32) / 8.0)[HEAD_PERM]

    qwT = _bf16(q_w_eff.T[:, HEAD_PERM].reshape(ND, 128, H * HD))
    kwT = _bf16(k_w_eff.T.reshape(ND, 128, KV * HD))
    vwT = _bf16(v_w_eff.T.reshape(ND, 128, KV * HD))
    owT = _bf16(np.asarray(o_w, np.float32).T[HEAD_PERM, :].reshape(NQC, 128, D))
    gwa = _bf16(np.ascontiguousarray(
        g_w_eff.reshape(NFC, 128, ND, 128).transpose(0, 3, 2, 1)))
    uwa = _bf16(np.ascontiguousarray(
        u_w_eff.reshape(NFC, 128, ND, 128).transpose(0, 3, 2, 1)))
    dwa = _bf16(np.ascontiguousarray(
        np.asarray(down_w, np.float32).reshape(ND, 128, NFC, 128).transpose(0, 3, 2, 1)))

    qb_a = np.ascontiguousarray(q_b_eff.reshape(NQC, 128).T).astype(np.float32)
    kb_a = np.ascontiguousarray(np.asarray(k_b, np.float32).reshape(NKC, 128).T)
    vb_a = np.broadcast_to(np.asarray(v_b, np.float32)[None, :], (128, KV * HD)).copy()
    tvals = (np.arange(NT)[None, :] * 128 + np.arange(128)[:, None]).astype(np.float32)

    shared = dict(qwT=qwT, kwT=kwT, vwT=vwT, owT=owT, gw=gwa, uw=uwa, dw=dwa,
                  qb=qb_a, kb=kb_a, vb=vb_a, tvals=tvals,
                  rope_m=_bf16(_rope_matrix()))

    def stack2(mat):        # [n, 64] -> [128, n] (head-pair stacked transpose)
        mT = mat.T.astype(np.float32)
        return np.concatenate([mT, mT], axis=0)

    in_maps = []
    for c in range(NCORES):
        b = c // 2
        pos = core_pos[c]
        im = dict(shared)
        im.update(
            hiddenT=_bf16(hidden_states[b].T.reshape(ND, 128, T)),
            selresT=np.ascontiguousarray(
                hidden_states[b][pos].T.reshape(ND, 128, NQ)).astype(np.float32),
            cos_q=_bf16(stack2(cos[b][pos])),
            sin_q=_bf16(stack2(sin[b][pos])),
            cos_k=_bf16(stack2(cos[b])),
            sin_k=_bf16(stack2(sin[b])),
            posq=np.broadcast_to(pos.astype(np.float32)[None, :], (128, NQ)).copy(),
            gmul=np.broadcast_to(gsc[b, c % 2::2].astype(np.float32)[None, :],
                                 (128, NQ)).copy(),
            selg=np.ascontiguousarray(
                (hidden_states[b][pos] * (1.0 - gsc[b, c % 2::2])[:, None])
                .T.reshape(ND, 128, NQ)).astype(np.float32),
        )
        in_maps.append(im)

    if _profile:
        _install_ntff_hook()
    res = run_bass_kernel_spmd(nc, in_maps, core_ids=list(range(NCORES)),
                               trace=_profile)

    out = hidden_states.copy()
    for c in range(NCORES):
        b = c // 2
        upd = res.results[c]["updT"].reshape(D, NQ).T
        out[b, core_pos[c], :] = upd
    if _profile or _dbg:
        return out, res
    return out



# revision 3
# speedup vs baseline: 1.1583x; 1.1583x over previous
"""Trainium2 Bass kernel for nn_DynamicBlock (sparse-token attention + MLP block).

Contract: kernel(**inputs) takes the FULL unsharded inputs (as produced by
reference.setup_inputs()) and returns the FULL [B, T, D] output.

Sharding: 8 cores = 4 batches x 2 interleaved query-halves. Each core:
 - computes rmsnorm + K/V projections (+rope on K) for its batch over all T,
 - processes its 256 selected queries: Q proj + rope, causal attention over
   all T keys (GQA 16 q-heads / 8 kv-heads), o-proj, MLP, gated update,
 - returns the 256 updated rows; the host scatters them into a copy of
   hidden_states.

v2: all projections (q/k/v/o/gate/up/down) run in fp8e4m3 with DoubleRow
perf mode (256-deep contraction per pass, 2x PE throughput). Weights are
scaled x64 (x256 for q incl. softmax scale, x32 for up) on the host to
clear the e4m3 subnormal range; dequant factors fold into the existing
bias/residual fused ops. Attention scores/ctx stay bf16. Causal masks are
precomputed on the host (one DMA) instead of per-tile is_ge ops. DMAs are
consolidated into a few large contiguous transfers with host-side layouts
matching SBUF. The attention t-loop is software-pipelined (scores of tile
t+1 issue before ctx of tile t) so PE doesn't stall on the exp/mask chain.
"""

import sys

sys.path.insert(0, "/opt/trn_rl_repo")

import numpy as np
import ml_dtypes

import concourse.bass as bass
import concourse.tile as tile
from concourse import mybir
from concourse.bass_utils import run_bass_kernel_spmd
from concourse.vector_clock import ScopedClock, VectorClock

BF16 = mybir.dt.bfloat16
F32 = mybir.dt.float32
F8 = mybir.dt.float8e4
AF = mybir.ActivationFunctionType
OP = mybir.AluOpType
DR = mybir.MatmulPerfMode.DoubleRow

B, T, D = 4, 2048, 1024
H, KV, HD = 16, 8, 64
DFF = 4096
KSEL = 512
EPS = 1e-6

NQ = 256          # queries per core
ND = D // 128     # 8 d-tiles
NT = T // 128     # 16 key tiles
NKC = KV * HD // 128   # 4 k-output chunks (2 kv heads each)
NQC = H * HD // 128    # 8 q-output chunks (2 q heads each)
NFC = DFF // 128       # 32 ff chunks
NCORES = 8

SQ = 256.0   # q weight scale (includes 1/8 softmax scale)
SK = 64.0
SV = 64.0
SO = 64.0
SG = 64.0
SU = 32.0
SD = 64.0
SMLP = SU * SD  # dequant for down-proj output

# q-head layout: q-chunk tile 2c holds heads (4c, 4c+2) on partition halves
# (kv heads 2c / 2c+1), tile 2c+1 holds (4c+1, 4c+3). kv head of q-head h is h//2.
TILE_HEADS = []
for c in range(4):
    TILE_HEADS.append((4 * c, 4 * c + 2))
    TILE_HEADS.append((4 * c + 1, 4 * c + 3))
HEAD_PERM = np.array([h * HD + i for pair in TILE_HEADS for h in pair for i in range(HD)])


# ---------------------------------------------------------------------------
# walrus workarounds: this toolchain encodes at most ONE semaphore wait per
# instruction. Split the tile tail-drain into per-proc drains and move excess
# waits onto NoOps.
# ---------------------------------------------------------------------------

def _patched_drain_and_barrier(self, tick_clock, wait_clock):
    gc = tick_clock.global_clock
    n = len(gc)
    for i in range(n):
        t = gc[i]
        if t > 0:
            vec = [0] * n
            vec[i] = t
            d = self.nc.sync.drain()
            wait_clock.add_sem_waits(d.ins, ScopedClock({None: VectorClock(vec)}))
    self.nc.all_engine_barrier()
    popped = self.nc._tile_sem_poison_stack.pop()
    assert popped is self._sem_poison
    self.nc.clear_and_free_semaphores(list(self.sems.allocated().values()))
    self.nc.all_engine_barrier()


tile.TileContext._drain_and_barrier = _patched_drain_and_barrier

_MAX_WAITS = 1


def _split_excess_waits(nc):
    for f in nc.m.functions:
        for bb in f.blocks:
            new = []
            for inst in bb.instructions:
                si = inst.sync_info
                if si is not None and si.on_wait is not None and len(si.on_wait) > _MAX_WAITS:
                    waits = list(si.on_wait)
                    excess, keep = waits[:-_MAX_WAITS], waits[-_MAX_WAITS:]
                    k = 0
                    while excess:
                        chunk, excess = excess[:_MAX_WAITS], excess[_MAX_WAITS:]
                        new.append(mybir.InstNoOp(
                            name=f"{inst.name}_ws{k}",
                            engine=inst.engine,
                            sync_info=mybir.SyncInfo(on_wait=chunk, on_update=[])))
                        k += 1
                    inst.sync_info = mybir.SyncInfo(
                        on_wait=keep, on_update=list(si.on_update or []))
                new.append(inst)
            bb.instructions = new


def _bcast_mid(ap_2d, n):
    """[P, W] AP -> [P, n(bcast), W] via a stride-0 middle dim."""
    return bass.AP(tensor=ap_2d.tensor, offset=ap_2d.offset,
                   ap=[ap_2d.ap[0], [0, n], ap_2d.ap[1]])


# ---------------------------------------------------------------------------
# device program
# ---------------------------------------------------------------------------

def build_program(qlo, qhi, dbg=False):
    """qlo/qhi: per key-tile [NT] compile-time query ranges (uniform across cores).

    For key tile tt only queries [qlo[tt]:NQ) attend any of its keys; queries in
    [qlo[tt]:qhi[tt]) are partially masked, [qhi[tt]:NQ) fully valid.
    """
    nc = bass.Bass(trn_type="TRN2", target_bir_lowering=False, debug=False)

    def inp(name, shape, dt):
        return nc.dram_tensor(name, shape, dt, kind="ExternalInput").ap()

    selres = inp("selres", [128, ND, NQ], F32)
    hidc = inp("hidc", [4, 128, ND, 512], BF16)
    qw8 = inp("qw8", [128, ND, H * HD], F8)
    kw8 = inp("kw8", [128, ND, KV * HD], F8)
    vw8 = inp("vw8", [128, ND, KV * HD], F8)
    ow8 = inp("ow8", [128, NQC, D], F8)
    gw8 = inp("gw8", [128, NFC, ND, 128], F8)
    uw8 = inp("uw8", [128, NFC, ND, 128], F8)
    dw8 = inp("dw8", [128, ND, NFC, 128], F8)
    # consts: qb[NQC] kb[NKC] vb[512] g[NQ] g_sc[NQ]
    NCONST = NQC + NKC + KV * HD + NQ + NQ
    consts = inp("consts", [128, NCONST], F32)
    rope_m = inp("rope_m", [128, 128], BF16)
    csq = inp("csq", [128, 2, NQ], BF16)     # [cos_q; sin_q]
    csk = inp("csk", [128, 2, T], BF16)      # [cos_k; sin_k]
    maskq = inp("maskq", [128, NT, NQ], BF16)
    selg = inp("selg", [128, ND, NQ], F32)   # selresT * (1 - g)

    updT = nc.dram_tensor("updT", [128, ND, NQ], F32, kind="ExternalOutput").ap()
    dbg_o = {}
    if dbg:
        for nm, shp, dt_ in [("d_normT", [128, ND, T], F8),
                             ("d_kT", [128, NKC, T], BF16),
                             ("d_vplus", [128, NT, KV, HD + 2], BF16),
                             ("d_qrT", [128, NQC, NQ], BF16),
                             ("d_ctxT", [128, NQC, NQ], F8),
                             ("d_hTt", [128, ND, NQ], F32),
                             ("d_n2T", [128, ND, NQ], F8),
                             ("d_actT", [128, NFC, NQ], F8)]:
            dbg_o[nm] = nc.dram_tensor(nm, shp, dt_, kind="ExternalOutput").ap()

    with tile.TileContext(nc, pool_alloc_mode="queue") as tc:
        with tc.tile_pool(name="ps", bufs=8, space="PSUM") as ps, \
             tc.tile_pool(name="persist", bufs=1) as pp, \
             tc.tile_pool(name="rows", bufs=2) as rowp, \
             tc.tile_pool(name="raw", bufs=2) as rawp:

            # ---- persistent tiles ------------------------------------------------
            selT = pp.tile([128, ND, NQ], F32, name="selT")
            nselT = pp.tile([128, ND, NQ], F8, name="nselT")
            qrT = pp.tile([128, NQC, NQ], BF16, name="qrT")
            normT8 = pp.tile([128, ND, T], F8, name="normT8")
            kT = pp.tile([128, NKC, T], BF16, name="kT")
            vplus = pp.tile([128, NT, KV, HD + 2], BF16, name="vplus")
            ctxT = pp.tile([128, NQC, NQ], F8, name="ctxT")
            hTt = pp.tile([128, ND, NQ], F32, name="hTt")
            n2T = pp.tile([128, ND, NQ], F8, name="n2T")
            actT = pp.tile([128, NFC, NQ], F8, name="actT")

            w_q = pp.tile([128, ND, H * HD], F8, name="w_q")
            w_k = pp.tile([128, ND, KV * HD], F8, name="w_k")
            w_v = pp.tile([128, ND, KV * HD], F8, name="w_v")
            w_o = pp.tile([128, NQC, D], F8, name="w_o")
            selgT = pp.tile([128, ND, NQ], F32, name="selgT")

            c_const = pp.tile([128, NCONST], F32, name="c_const")
            c_qb = c_const[:, 0:NQC]
            c_kb = c_const[:, NQC:NQC + NKC]
            c_vb = c_const[:, NQC + NKC:NQC + NKC + KV * HD]
            OG = NQC + NKC + KV * HD
            c_g = c_const[:, OG:OG + NQ]
            c_gsc = c_const[:, OG + NQ:OG + 2 * NQ]
            c_rm = pp.tile([128, 128], BF16, name="c_rm")
            c_csq = pp.tile([128, 2, NQ], BF16, name="c_csq")
            c_csk = pp.tile([128, 2, T], BF16, name="c_csk")
            c_mask = pp.tile([128, NT, NQ], BF16, name="c_mask")

            ones_t = pp.tile([128, 1], BF16, name="ones_t")
            nc.vector.memset(ones_t, 1.0)
            eps_t = pp.tile([1, 1], F32, name="eps_t")
            nc.vector.memset(eps_t, EPS)
            ones_all = pp.tile([128, 128], F32, name="ones_all")
            nc.vector.memset(ones_all, 1.0)
            nc.vector.memset(vplus[:, :, :, 0:1], 1.0)
            nc.vector.memset(vplus[:, :, :, HD + 1:HD + 2], 1.0)

            # ---- front DMAs (order per queue = emission order) -------------------
            nc.sync.dma_start(out=selT, in_=selres)
            nc.scalar.dma_start(out=c_const, in_=consts)
            nc.scalar.dma_start(out=c_csq, in_=csq)
            nc.scalar.dma_start(out=c_rm, in_=rope_m)
            nc.gpsimd.dma_start(out=w_q, in_=qw8)
            nc.gpsimd.dma_start(out=w_k, in_=kw8)
            nc.gpsimd.dma_start(out=w_v, in_=vw8)

            raw_t = [None] * 4
            raw_t[0] = rawp.tile([128, ND, 512], BF16, name="raw0", tag="raw")
            nc.sync.dma_start(out=raw_t[0], in_=hidc[0])
            raw_t[1] = rawp.tile([128, ND, 512], BF16, name="raw1", tag="raw")
            nc.scalar.dma_start(out=raw_t[1], in_=hidc[1])
            nc.scalar.dma_start(out=c_csk, in_=csk)
            nc.gpsimd.dma_start(out=c_mask, in_=maskq)
            nc.gpsimd.dma_start(out=w_o, in_=ow8)
            nc.gpsimd.dma_start(out=selgT, in_=selg)

            # ======================================================================
            # Phase 3 (emitted first): selected-row rmsnorm + Q proj + rope
            # ======================================================================
            with tc.tile_pool(name="ph3", bufs=3) as p3:
                sq3 = p3.tile([128, ND, NQ], BF16, name="sq3", bufs=1)
                nc.vector.tensor_mul(out=sq3, in0=selT, in1=selT)
                ssq = ps.tile([128, 512], F32, name="ssq", tag="ps")
                for dt in range(ND):
                    nc.tensor.matmul(ssq[0:1, 0:NQ], lhsT=ones_t, rhs=sq3[:, dt, :],
                                     start=(dt == 0), stop=(dt == ND - 1))
                srow3 = rowp.tile([1, NQ], F32, name="srow3", tag="row")
                nc.scalar.activation(out=srow3, in_=ssq[0:1, 0:NQ], func=AF.Sqrt,
                                     bias=eps_t[0:1, 0:1], scale=1.0 / D)
                rrow3 = rowp.tile([1, NQ], F32, name="rrow3", tag="row")
                nc.vector.reciprocal(out=rrow3, in_=srow3)
                rbc3 = ps.tile([128, 512], F32, name="rbc3", tag="ps")
                nc.tensor.matmul(rbc3[:, 0:NQ], lhsT=ones_all[0:1, :], rhs=rrow3,
                                 start=True, stop=True)
                rbc3_sb = p3.tile([128, NQ], F32, name="rbc3_sb", bufs=1)
                nc.vector.tensor_copy(out=rbc3_sb, in_=rbc3[:, 0:NQ])
                nc.vector.tensor_mul(out=nselT, in0=selT,
                                     in1=_bcast_mid(rbc3_sb, ND))

                for qc in range(NQC):
                    qps = ps.tile([128, 512], F32, name="qps", tag="ps")
                    for d2 in range(ND // 2):
                        nc.tensor.matmul(
                            qps[:, 0:NQ],
                            lhsT=w_q[:, 2 * d2:2 * d2 + 2, qc * 128:(qc + 1) * 128],
                            rhs=nselT[:, 2 * d2:2 * d2 + 2, :],
                            start=(d2 == 0), stop=(d2 == ND // 2 - 1),
                            perf_mode=DR)
                    qraw = p3.tile([128, NQ], BF16, name="qraw")
                    nc.vector.tensor_scalar(
                        out=qraw, in0=qps[:, 0:NQ], scalar1=1.0 / SQ,
                        scalar2=c_qb[:, qc:qc + 1], op0=OP.mult, op1=OP.add)
                    rotq = ps.tile([128, 512], F32, name="rotq", tag="ps")
                    nc.tensor.matmul(rotq[:, 0:NQ], lhsT=c_rm, rhs=qraw,
                                     start=True, stop=True)
                    dst = qrT[:, qc, :]
                    tmpq = p3.tile([128, NQ], BF16, name="tmpq")
                    nc.vector.tensor_mul(out=tmpq, in0=rotq[:, 0:NQ],
                                         in1=c_csq[:, 1, :])
                    nc.vector.tensor_mul(out=dst, in0=qraw, in1=c_csq[:, 0, :])
                    nc.vector.tensor_add(out=dst, in0=dst, in1=tmpq)

            # ======================================================================
            # Phases 1+2 fused, chunk-major: per 512-token chunk compute rmsnorm,
            # then K (+rope) and V for that chunk so PE work stays dense.
            # ======================================================================
            with tc.tile_pool(name="ph1", bufs=3) as p1, \
                 tc.tile_pool(name="ph2", bufs=3) as p2:
                for ch in range(4):
                    if ch + 2 < 4:
                        raw_t[ch + 2] = rawp.tile([128, ND, 512], BF16,
                                                  name=f"raw{ch + 2}", tag="raw")
                        eng = nc.sync if (ch + 2) % 2 == 0 else nc.scalar
                        eng.dma_start(out=raw_t[ch + 2], in_=hidc[ch + 2])
                    raw = raw_t[ch]
                    cs = slice(ch * 512, (ch + 1) * 512)
                    sq = p1.tile([128, ND, 512], BF16, name="sq", bufs=2)
                    nc.vector.tensor_mul(out=sq, in0=raw, in1=raw)
                    ssp = ps.tile([128, 512], F32, name="ssp", tag="ps")
                    for dt in range(ND):
                        nc.tensor.matmul(ssp[0:1, :], lhsT=ones_t, rhs=sq[:, dt, :],
                                         start=(dt == 0), stop=(dt == ND - 1))
                    srow = rowp.tile([1, 512], F32, name="srow", tag="row")
                    nc.scalar.activation(out=srow, in_=ssp[0:1, :], func=AF.Sqrt,
                                         bias=eps_t[0:1, 0:1], scale=1.0 / D)
                    rrow = rowp.tile([1, 512], F32, name="rrow", tag="row")
                    nc.vector.reciprocal(out=rrow, in_=srow)
                    rbc = ps.tile([128, 512], F32, name="rbc", tag="ps")
                    nc.tensor.matmul(rbc, lhsT=ones_all[0:1, :], rhs=rrow,
                                     start=True, stop=True)
                    rbc_sb = p1.tile([128, 512], BF16, name="rbc_sb", bufs=2)
                    nc.vector.tensor_copy(out=rbc_sb, in_=rbc)
                    nc.vector.tensor_mul(out=normT8[:, :, cs], in0=raw,
                                         in1=_bcast_mid(rbc_sb, ND))

                    # K for this chunk
                    for kc in range(NKC):
                        kps = ps.tile([128, 512], F32, name="kps", tag="ps")
                        for d2 in range(ND // 2):
                            nc.tensor.matmul(
                                kps,
                                lhsT=w_k[:, 2 * d2:2 * d2 + 2, kc * 128:(kc + 1) * 128],
                                rhs=normT8[:, 2 * d2:2 * d2 + 2, cs],
                                start=(d2 == 0), stop=(d2 == ND // 2 - 1),
                                perf_mode=DR)
                        kraw = p2.tile([128, 512], BF16, name="kraw")
                        nc.vector.tensor_scalar(
                            out=kraw, in0=kps, scalar1=1.0 / SK,
                            scalar2=c_kb[:, kc:kc + 1], op0=OP.mult, op1=OP.add)
                        rot = ps.tile([128, 512], F32, name="rot", tag="ps")
                        nc.tensor.matmul(rot, lhsT=c_rm, rhs=kraw,
                                         start=True, stop=True)
                        dst = kT[:, kc, cs]
                        tmp = p2.tile([128, 512], BF16, name="tmp")
                        nc.vector.tensor_mul(out=tmp, in0=rot, in1=c_csk[:, 1, cs])
                        nc.vector.tensor_mul(out=dst, in0=kraw, in1=c_csk[:, 0, cs])
                        nc.vector.tensor_add(out=dst, in0=dst, in1=tmp)

                    # V for this chunk's 4 key tiles
                    for tt in range(ch * 4, ch * 4 + 4):
                        vps = ps.tile([128, 512], F32, name="vps", tag="ps")
                        for d2 in range(ND // 2):
                            nc.tensor.matmul(
                                vps,
                                lhsT=normT8[:, 2 * d2:2 * d2 + 2,
                                            tt * 128:(tt + 1) * 128],
                                rhs=w_v[:, 2 * d2:2 * d2 + 2, :],
                                start=(d2 == 0), stop=(d2 == ND // 2 - 1),
                                perf_mode=DR)
                        nc.vector.scalar_tensor_tensor(
                            out=vplus[:, tt, :, 1:HD + 1],
                            in0=vps.rearrange("p (h d) -> p h d", h=KV),
                            scalar=1.0 / SV,
                            in1=c_vb.rearrange("p (h d) -> p h d", h=KV),
                            op0=OP.mult, op1=OP.add)

            if dbg:
                nc.scalar.dma_start(out=dbg_o["d_normT"], in_=normT8)
                nc.scalar.dma_start(out=dbg_o["d_kT"], in_=kT)
                nc.scalar.dma_start(out=dbg_o["d_vplus"], in_=vplus)
                nc.scalar.dma_start(out=dbg_o["d_qrT"], in_=qrT)

            # ======================================================================
            # Phase 4: attention. 2 groups x 2 kv-chunks; per key tile: scores
            # (K=64 row-group pairs), exp, partial causal mask, ctx accumulate.
            # ctx psum pair tile packs 2 heads along free dim; all at parts 0:65.
            # Software-pipelined at two levels: scores of tile t+1 are emitted
            # before ctx of tile t (PE never waits on the exp/mask chain), and
            # group kc+1's t-loop is emitted before group kc's eviction.
            # ======================================================================
            with tc.tile_pool(name="ph4", bufs=1) as p4:
                live = [t_ for t_ in range(NT) if qlo[t_] < NQ]
                last_tt = max(live)
                cps_all = {}

                def attn_tloop(kc):
                    for ab in range(2):
                        cps_all[(kc, ab)] = ps.tile([128, 512], F32,
                                                    name=f"cps{kc}{ab}", tag="ps")

                    pts = {}

                    def emit_scores(tt):
                        lo = qlo[tt]
                        for half in range(2):
                            # one bank holds the same row-group half of both
                            # q-tiles (A, B) -> same-bank PE writes stay
                            # serial; cross-bank halves run concurrently.
                            hs_ = slice(half * 64, (half + 1) * 64)
                            sp = ps.tile([128, 512], F32, name="sp", tag="ps")
                            for ab in range(2):
                                nc.tensor.matmul(
                                    sp[:, ab * NQ + lo:ab * NQ + NQ],
                                    lhsT=kT[hs_, kc, tt * 128:(tt + 1) * 128],
                                    rhs=qrT[hs_, 2 * kc + ab, lo:NQ],
                                    start=(ab == 0), stop=(ab == 1))
                            pt = p4.tile([128, 2, NQ], BF16, name="pt", bufs=8)
                            nc.scalar.activation(
                                out=pt[:, :, lo:NQ],
                                in_=sp.rearrange("p (h q) -> p h q", h=2)[:, :, lo:NQ],
                                func=AF.Exp)
                            hi = qhi[tt]
                            if hi > lo:
                                nc.vector.tensor_mul(
                                    out=pt[:, :, lo:hi],
                                    in0=pt[:, :, lo:hi],
                                    in1=_bcast_mid(c_mask[:, tt, lo:hi], 2))
                            pts[(tt, half)] = (sp, pt)

                    def emit_ctx(tt):
                        lo = qlo[tt]
                        for half in range(2):
                            sp, pt = pts.pop((tt, half))
                            kvh = 2 * kc + half
                            for ab in range(2):
                                cp = cps_all[(kc, ab)]
                                # start/stop once per PSUM BANK (zero region)
                                nc.tensor.matmul(
                                    cp[0:HD + 1, half * NQ + lo:half * NQ + NQ],
                                    lhsT=vplus[:, tt, kvh, 1:HD + 2],
                                    rhs=pt[:, ab, lo:NQ],
                                    start=(tt == live[0] and half == 0),
                                    stop=(tt == last_tt and half == 1))

                    for i, tt in enumerate(live):
                        emit_scores(tt)
                        if i >= 1:
                            emit_ctx(live[i - 1])
                    emit_ctx(live[-1])

                def attn_evict(kc):
                    # scale by 1/rowsum; odd halves relocated to partitions
                    # 64:128 via SBUF->SBUF DMA (DVE can't cross partitions)
                    for ab in range(2):
                        cp = cps_all[(kc, ab)]
                        rr = p4.tile([128, 512], F32, name="rr", bufs=2)
                        nc.vector.reciprocal(out=rr[64:65, :],
                                             in_=cp[HD:HD + 1, :])
                        rb = ps.tile([128, 512], F32, name="rb", tag="ps")
                        nc.tensor.matmul(rb[0:64, :],
                                         lhsT=ones_all[64:65, 0:64],
                                         rhs=rr[64:65, :],
                                         start=True, stop=True)
                        rb_sb = p4.tile([64, 512], F32, name="rb_sb", bufs=2)
                        nc.vector.tensor_copy(out=rb_sb, in_=rb[0:64, :])
                        nc.vector.tensor_mul(
                            out=ctxT[0:64, 2 * kc + ab, :],
                            in0=cp[0:HD, 0:NQ], in1=rb_sb[:, 0:NQ])
                        stage = p4.tile([64, NQ], F8, name="stage", bufs=2)
                        nc.vector.tensor_mul(
                            out=stage, in0=cp[0:HD, NQ:2 * NQ],
                            in1=rb_sb[:, NQ:2 * NQ])
                        nc.sync.dma_start(
                            out=ctxT[64:128, 2 * kc + ab, :], in_=stage)

                # software-pipelined: next group's t-loop is emitted before the
                # previous group's eviction so PE never waits on evictions.
                attn_tloop(0)
                attn_tloop(1)
                attn_evict(0)
                attn_tloop(2)
                attn_evict(1)
                attn_tloop(3)
                attn_evict(2)
                attn_evict(3)

            # ======================================================================
            # Phase 5: o-proj + residual -> hTt (fp32)
            # ======================================================================
            with tc.tile_pool(name="ph5", bufs=2) as p5:
                for dc in range(ND):
                    ops_ = ps.tile([128, 512], F32, name="ops_", tag="ps")
                    for h2 in range(NQC // 2):
                        nc.tensor.matmul(
                            ops_[:, 0:NQ],
                            lhsT=w_o[:, 2 * h2:2 * h2 + 2, dc * 128:(dc + 1) * 128],
                            rhs=ctxT[:, 2 * h2:2 * h2 + 2, :],
                            start=(h2 == 0), stop=(h2 == NQC // 2 - 1),
                            perf_mode=DR)
                    nc.vector.scalar_tensor_tensor(
                        out=hTt[:, dc, :], in0=ops_[:, 0:NQ], scalar=1.0 / SO,
                        in1=selT[:, dc, :], op0=OP.mult, op1=OP.add)

            if dbg:
                nc.scalar.dma_start(out=dbg_o["d_ctxT"], in_=ctxT)
                nc.scalar.dma_start(out=dbg_o["d_hTt"], in_=hTt)

            # ======================================================================
            # Phase 6: rmsnorm2 -> n2T (fp8); then hTt := g*hTt + selg (the
            # gated-residual part that phase 8 adds to the scaled mps).
            # ======================================================================
            with tc.tile_pool(name="ph6", bufs=3) as p6:
                sq6 = p6.tile([128, ND, NQ], BF16, name="sq6", bufs=1)
                nc.vector.tensor_mul(out=sq6, in0=hTt, in1=hTt)
                ss2 = ps.tile([128, 512], F32, name="ss2", tag="ps")
                for dt in range(ND):
                    nc.tensor.matmul(ss2[0:1, 0:NQ], lhsT=ones_t, rhs=sq6[:, dt, :],
                                     start=(dt == 0), stop=(dt == ND - 1))
                srow6 = rowp.tile([1, NQ], F32, name="srow6", tag="row")
                nc.scalar.activation(out=srow6, in_=ss2[0:1, 0:NQ], func=AF.Sqrt,
                                     bias=eps_t[0:1, 0:1], scale=1.0 / D)
                rrow6 = rowp.tile([1, NQ], F32, name="rrow6", tag="row")
                nc.vector.reciprocal(out=rrow6, in_=srow6)
                rbc6 = ps.tile([128, 512], F32, name="rbc6", tag="ps")
                nc.tensor.matmul(rbc6[:, 0:NQ], lhsT=ones_all[0:1, :], rhs=rrow6,
                                 start=True, stop=True)
                rbc6_sb = p6.tile([128, NQ], F32, name="rbc6_sb", bufs=1)
                nc.vector.tensor_copy(out=rbc6_sb, in_=rbc6[:, 0:NQ])
                nc.vector.tensor_mul(out=n2T, in0=hTt,
                                     in1=_bcast_mid(rbc6_sb, ND))
                nc.vector.tensor_mul(out=hTt, in0=hTt, in1=_bcast_mid(c_g, ND))
                nc.vector.tensor_add(out=hTt, in0=hTt, in1=selgT)

            if dbg:
                nc.scalar.dma_start(out=dbg_o["d_n2T"], in_=n2T)

            # ======================================================================
            # Phase 7: MLP gate/up (fp8 DoubleRow) -> actT (fp8)
            # ======================================================================
            with tc.tile_pool(name="ph7w", bufs=3) as p7w, \
                 tc.tile_pool(name="ph7", bufs=3) as p7:
                NGRP = 4
                for g0 in range(0, NFC, NGRP):
                    wg_t = p7w.tile([128, NGRP, ND, 128], F8, name="wg_t")
                    nc.sync.dma_start(out=wg_t, in_=gw8[:, g0:g0 + NGRP])
                    wu_t = p7w.tile([128, NGRP, ND, 128], F8, name="wu_t")
                    nc.scalar.dma_start(out=wu_t, in_=uw8[:, g0:g0 + NGRP])
                    for j in range(NGRP):
                        fc = g0 + j
                        gps = ps.tile([128, 512], F32, name="gps", tag="ps")
                        ups = ps.tile([128, 512], F32, name="ups", tag="ps")
                        for d2 in range(ND // 2):
                            nc.tensor.matmul(
                                gps[:, 0:NQ], lhsT=wg_t[:, j, 2 * d2:2 * d2 + 2, :],
                                rhs=n2T[:, 2 * d2:2 * d2 + 2, :],
                                start=(d2 == 0), stop=(d2 == ND // 2 - 1),
                                perf_mode=DR)
                        for d2 in range(ND // 2):
                            nc.tensor.matmul(
                                ups[:, 0:NQ], lhsT=wu_t[:, j, 2 * d2:2 * d2 + 2, :],
                                rhs=n2T[:, 2 * d2:2 * d2 + 2, :],
                                start=(d2 == 0), stop=(d2 == ND // 2 - 1),
                                perf_mode=DR)
                        sg = p7.tile([128, NQ], BF16, name="sg")
                        nc.scalar.activation(out=sg, in_=gps[:, 0:NQ], func=AF.Silu,
                                             scale=1.0 / SG)
                        nc.vector.tensor_mul(out=actT[:, fc, :], in0=ups[:, 0:NQ],
                                             in1=sg)

            if dbg:
                nc.scalar.dma_start(out=dbg_o["d_actT"], in_=actT)

            # ======================================================================
            # Phase 8: down-proj (fp8 DoubleRow) + gated residual update.
            # f1 written into selT (free by now); one output DMA at the end.
            # ======================================================================
            with tc.tile_pool(name="ph8w", bufs=2) as p8w:
                for dc in range(ND):
                    wd_t = p8w.tile([128, NFC, 128], F8, name="wd_t")
                    (nc.sync if dc % 2 == 0 else nc.gpsimd).dma_start(
                        out=wd_t, in_=dw8[:, dc])
                    mps = ps.tile([128, 512], F32, name="mps", tag="ps")
                    for f2 in range(NFC // 2):
                        nc.tensor.matmul(
                            mps[:, 0:NQ], lhsT=wd_t[:, 2 * f2:2 * f2 + 2, :],
                            rhs=actT[:, 2 * f2:2 * f2 + 2, :],
                            start=(f2 == 0), stop=(f2 == NFC // 2 - 1),
                            perf_mode=DR)
                    # updated = (selg + g*h) + g*mlp ; mps = SMLP*mlp
                    dst = selT[:, dc, :]
                    nc.vector.tensor_mul(out=dst, in0=mps[:, 0:NQ], in1=c_gsc)
                    nc.vector.tensor_add(out=dst, in0=dst, in1=hTt[:, dc, :])
                nc.gpsimd.dma_start(out=updT, in_=selT)

    _split_excess_waits(nc)
    return nc


# ---------------------------------------------------------------------------
# host side
# ---------------------------------------------------------------------------

def _bf16(x):
    return np.asarray(x, dtype=np.float32).astype(ml_dtypes.bfloat16)


def _f8(x):
    return np.asarray(x, dtype=np.float32).astype(ml_dtypes.float8_e4m3)


def _rope_matrix():
    """R[k, p] = sign(p) * 1[k == swap(p)]; (R.T @ x)[p] = sign(p)*x[swap(p)].

    rot(x)[p%64 < 32] = -x[p+32], else +x[p-32]  (two stacked 64-dim heads).
    """
    R = np.zeros((128, 128), np.float32)
    for p in range(128):
        base = (p // 64) * 64
        off = p % 64
        if off < 32:
            R[base + off + 32, p] = -1.0
        else:
            R[base + off - 32, p] = 1.0
    return R


def _install_ntff_hook():
    """Shim antenv.axon_hooks (absent in this image) so trace=True works."""
    import types
    try:
        import antenv.axon_hooks  # noqa: F401
        return
    except ImportError:
        pass
    try:
        from trn_agent_boot.trn_boot import _ntff_profile_via_ctypes
        hook = _ntff_profile_via_ctypes("/opt/axon/libaxon_pjrt.so")
    except Exception:
        hook = None
    mod = types.ModuleType("antenv.axon_hooks")
    mod._hook = hook
    mod.set_axon_ntff_profile_hook = lambda h: setattr(mod, "_hook", h)
    mod.get_axon_ntff_profile_hook = lambda: mod._hook
    sys.modules["antenv.axon_hooks"] = mod


def kernel(hidden_states, token_indices, batch_indices, gating_scores, cos, sin,
           ln1_w, ln2_w, q_w, q_b, k_w, k_b, v_w, v_b, o_w, gate_w, up_w, down_w,
           _profile=False, _dbg=False):
    hidden_states = np.asarray(hidden_states, dtype=np.float32)
    token_indices = np.asarray(token_indices).astype(np.int64)
    gating_scores = np.asarray(gating_scores, dtype=np.float32)
    cos = np.asarray(cos, dtype=np.float32)
    sin = np.asarray(sin, dtype=np.float32)
    ln1_w = np.asarray(ln1_w, dtype=np.float32)
    ln2_w = np.asarray(ln2_w, dtype=np.float32)

    topk = token_indices.reshape(B, KSEL)
    gsc = gating_scores.reshape(B, KSEL)

    core_pos = []
    for c in range(NCORES):
        b = c // 2
        core_pos.append(np.asarray(topk[b, c % 2::2], dtype=np.int64))

    qlo = [min(int(np.searchsorted(core_pos[c], tt * 128)) for c in range(NCORES))
           for tt in range(NT)]
    qhi = [max(int(np.searchsorted(core_pos[c], tt * 128 + 126, side="right"))
               for c in range(NCORES))
           for tt in range(NT)]

    nc = build_program(qlo, qhi, dbg=_dbg)

    # ---- weights (shared across cores) ----
    q_w_eff = (np.asarray(q_w, np.float32) * ln1_w[None, :]) * (SQ / 8.0)
    k_w_eff = np.asarray(k_w, np.float32) * ln1_w[None, :] * SK
    v_w_eff = np.asarray(v_w, np.float32) * ln1_w[None, :] * SV
    g_w_eff = np.asarray(gate_w, np.float32) * ln2_w[None, :] * SG
    u_w_eff = np.asarray(up_w, np.float32) * ln2_w[None, :] * SU
    q_b_eff = (np.asarray(q_b, np.float32) / 8.0)[HEAD_PERM]

    # layouts: [128, ND, out] with element [p, dt, m] = W_eff[m, dt*128+p]
    qw8 = _f8(np.ascontiguousarray(
        q_w_eff.T[:, HEAD_PERM].reshape(ND, 128, H * HD).transpose(1, 0, 2)))
    kw8 = _f8(np.ascontiguousarray(
        k_w_eff.T.reshape(ND, 128, KV * HD).transpose(1, 0, 2)))
    vw8 = _f8(np.ascontiguousarray(
        v_w_eff.T.reshape(ND, 128, KV * HD).transpose(1, 0, 2)))
    ow8 = _f8(np.ascontiguousarray(
        (np.asarray(o_w, np.float32) * SO).T[HEAD_PERM, :]
        .reshape(NQC, 128, D).transpose(1, 0, 2)))
    gw8 = _f8(np.ascontiguousarray(
        g_w_eff.reshape(NFC, 128, ND, 128).transpose(3, 0, 2, 1)))
    uw8 = _f8(np.ascontiguousarray(
        u_w_eff.reshape(NFC, 128, ND, 128).transpose(3, 0, 2, 1)))
    dw8 = _f8(np.ascontiguousarray(
        (np.asarray(down_w, np.float32) * SD)
        .reshape(ND, 128, NFC, 128).transpose(3, 0, 2, 1)))

    qb_a = np.ascontiguousarray(q_b_eff.reshape(NQC, 128).T).astype(np.float32)
    kb_a = np.ascontiguousarray(np.asarray(k_b, np.float32).reshape(NKC, 128).T)
    vb_a = np.broadcast_to(np.asarray(v_b, np.float32)[None, :], (128, KV * HD))

    shared = dict(qw8=qw8, kw8=kw8, vw8=vw8, ow8=ow8, gw8=gw8, uw8=uw8, dw8=dw8,
                  rope_m=_bf16(_rope_matrix()))

    def stack2(a, b):       # two [n, 64] -> [128, 2, n]
        out = np.empty((128, 2, a.shape[0]), np.float32)
        aT = a.T; bT = b.T
        out[:64, 0] = aT; out[64:, 0] = aT
        out[:64, 1] = bT; out[64:, 1] = bT
        return out

    in_maps = []
    for c in range(NCORES):
        b = c // 2
        pos = core_pos[c]
        g_c = gsc[b, c % 2::2].astype(np.float32)
        consts = np.empty((128, NQC + NKC + KV * HD + NQ + NQ), np.float32)
        consts[:, 0:NQC] = qb_a
        consts[:, NQC:NQC + NKC] = kb_a
        consts[:, NQC + NKC:NQC + NKC + KV * HD] = vb_a
        OG = NQC + NKC + KV * HD
        consts[:, OG:OG + NQ] = g_c[None, :]
        consts[:, OG + NQ:OG + 2 * NQ] = (g_c / SMLP)[None, :]

        # causal masks per key tile: maskq[p, tt, q] = 1 if pos[q] >= tt*128+p
        key_abs = (np.arange(NT)[None, :, None] * 128
                   + np.arange(128)[:, None, None])          # [128, NT, 1]
        maskq = (pos[None, None, :] >= key_abs)              # [128, NT, NQ]

        hb = hidden_states[b]                                # [T, D]
        selres = np.ascontiguousarray(
            hb[pos].T.reshape(ND, 128, NQ).transpose(1, 0, 2)).astype(np.float32)
        selg = np.ascontiguousarray(
            (hb[pos] * (1.0 - g_c)[:, None]).T
            .reshape(ND, 128, NQ).transpose(1, 0, 2)).astype(np.float32)
        hidc = np.ascontiguousarray(
            _bf16(hb.T).reshape(ND, 128, 4, 512).transpose(2, 1, 0, 3))

        im = dict(shared)
        im.update(
            selres=selres,
            hidc=hidc,
            consts=consts,
            csq=_bf16(stack2(cos[b][pos], sin[b][pos])),
            csk=_bf16(stack2(cos[b], sin[b])),
            maskq=_bf16(maskq.astype(np.float32)),
            selg=selg,
        )
        in_maps.append(im)

    if _profile:
        _install_ntff_hook()
    res = run_bass_kernel_spmd(nc, in_maps, core_ids=list(range(NCORES)),
                               trace=_profile)

    out = hidden_states.copy()
    for c in range(NCORES):
        b = c // 2
        upd = res.results[c]["updT"].transpose(1, 0, 2).reshape(D, NQ).T
        out[b, core_pos[c], :] = upd
    if _profile or _dbg:
        return out, res
    return out


# revision 25
# speedup vs baseline: 1.3118x; 1.1325x over previous
"""Trainium2 Bass kernel for nn_DynamicBlock (sparse-token attention + MLP block).

Contract: kernel(**inputs) takes the FULL unsharded inputs (as produced by
reference.setup_inputs()) and returns the FULL [B, T, D] output.

Sharding: 8 cores = 4 batches x 2 interleaved query-halves. Each core:
 - computes rmsnorm + K/V projections (+rope on K) for its batch over all T,
 - processes its 256 selected queries: Q proj + rope, causal attention over
   all T keys (GQA 16 q-heads / 8 kv-heads), o-proj, MLP, gated update,
 - returns the 256 updated rows; the host scatters them into a copy of
   hidden_states.

v2: all projections (q/k/v/o/gate/up/down) run in fp8e4m3 with DoubleRow
perf mode (256-deep contraction per pass, 2x PE throughput). Weights are
scaled x64 (x256 for q incl. softmax scale, x32 for up) on the host to
clear the e4m3 subnormal range; dequant factors fold into the existing
bias/residual fused ops. Attention scores/ctx stay bf16. Causal masks are
precomputed on the host (one DMA) instead of per-tile is_ge ops. DMAs are
consolidated into a few large contiguous transfers with host-side layouts
matching SBUF. The attention t-loop is software-pipelined (scores of tile
t+1 issue before ctx of tile t) so PE doesn't stall on the exp/mask chain.
"""

import sys

sys.path.insert(0, "/opt/trn_rl_repo")

import numpy as np
import ml_dtypes

import concourse.bass as bass
import concourse.tile as tile
from concourse import mybir
from concourse.bass_utils import run_bass_kernel_spmd
from concourse.vector_clock import ScopedClock, VectorClock

BF16 = mybir.dt.bfloat16
F32 = mybir.dt.float32
F8 = mybir.dt.float8e4
AF = mybir.ActivationFunctionType
OP = mybir.AluOpType
DR = mybir.MatmulPerfMode.DoubleRow

B, T, D = 4, 2048, 1024
H, KV, HD = 16, 8, 64
DFF = 4096
KSEL = 512
EPS = 1e-6

NQ = 256          # queries per core
ND = D // 128     # 8 d-tiles
NT = T // 128     # 16 key tiles
NKC = KV * HD // 128   # 4 k-output chunks (2 kv heads each)
NQC = H * HD // 128    # 8 q-output chunks (2 q heads each)
NFC = DFF // 128       # 32 ff chunks
NCORES = 8

SQ = 256.0   # q weight scale (includes 1/8 softmax scale)
SK = 64.0
SV = 64.0
SO = 64.0
SG = 64.0
SU = 32.0
SD = 64.0
SMLP = SU * SD  # dequant for down-proj output

# q-head layout: q-chunk tile 2c holds heads (4c, 4c+2) on partition halves
# (kv heads 2c / 2c+1), tile 2c+1 holds (4c+1, 4c+3). kv head of q-head h is h//2.
TILE_HEADS = []
for c in range(4):
    TILE_HEADS.append((4 * c, 4 * c + 2))
    TILE_HEADS.append((4 * c + 1, 4 * c + 3))
HEAD_PERM = np.array([h * HD + i for pair in TILE_HEADS for h in pair for i in range(HD)])


# ---------------------------------------------------------------------------
# walrus workarounds: this toolchain encodes at most ONE semaphore wait per
# instruction. Split the tile tail-drain into per-proc drains and move excess
# waits onto NoOps.
# ---------------------------------------------------------------------------

def _patched_drain_and_barrier(self, tick_clock, wait_clock):
    gc = tick_clock.global_clock
    n = len(gc)
    for i in range(n):
        t = gc[i]
        if t > 0:
            vec = [0] * n
            vec[i] = t
            d = self.nc.sync.drain()
            wait_clock.add_sem_waits(d.ins, ScopedClock({None: VectorClock(vec)}))
    self.nc.all_engine_barrier()
    popped = self.nc._tile_sem_poison_stack.pop()
    assert popped is self._sem_poison
    self.nc.clear_and_free_semaphores(list(self.sems.allocated().values()))
    self.nc.all_engine_barrier()


tile.TileContext._drain_and_barrier = _patched_drain_and_barrier

_MAX_WAITS = 1


def _split_excess_waits(nc):
    for f in nc.m.functions:
        for bb in f.blocks:
            new = []
            for inst in bb.instructions:
                si = inst.sync_info
                if si is not None and si.on_wait is not None and len(si.on_wait) > _MAX_WAITS:
                    waits = list(si.on_wait)
                    excess, keep = waits[:-_MAX_WAITS], waits[-_MAX_WAITS:]
                    k = 0
                    while excess:
                        chunk, excess = excess[:_MAX_WAITS], excess[_MAX_WAITS:]
                        new.append(mybir.InstNoOp(
                            name=f"{inst.name}_ws{k}",
                            engine=inst.engine,
                            sync_info=mybir.SyncInfo(on_wait=chunk, on_update=[])))
                        k += 1
                    inst.sync_info = mybir.SyncInfo(
                        on_wait=keep, on_update=list(si.on_update or []))
                new.append(inst)
            bb.instructions = new


def _bcast_mid(ap_2d, n):
    """[P, W] AP -> [P, n(bcast), W] via a stride-0 middle dim."""
    return bass.AP(tensor=ap_2d.tensor, offset=ap_2d.offset,
                   ap=[ap_2d.ap[0], [0, n], ap_2d.ap[1]])


# ---------------------------------------------------------------------------
# device program
# ---------------------------------------------------------------------------

def build_program(qlo, qhi, dbg=False):
    """qlo/qhi: per key-tile [NT] compile-time query ranges (uniform across cores).

    For key tile tt only queries [qlo[tt]:NQ) attend any of its keys; queries in
    [qlo[tt]:qhi[tt]) are partially masked, [qhi[tt]:NQ) fully valid.
    """
    nc = bass.Bass(trn_type="TRN2", target_bir_lowering=False, debug=False)

    def inp(name, shape, dt):
        return nc.dram_tensor(name, shape, dt, kind="ExternalInput").ap()

    selres = inp("selres", [128, ND, NQ], F32)
    hidc = inp("hidc", [4, 128, ND, 512], BF16)
    qw8 = inp("qw8", [128, ND, H * HD], F8)
    kw8 = inp("kw8", [128, ND, KV * HD], F8)
    vw8 = inp("vw8", [128, ND, KV * HD], F8)
    ow8 = inp("ow8", [128, NQC, D], F8)
    gw8 = inp("gw8", [128, NFC, ND, 128], F8)
    uw8 = inp("uw8", [128, NFC, ND, 128], F8)
    dw8 = inp("dw8", [128, ND, NFC, 128], F8)
    # consts: qb[NQC] kb[NKC] vb[512] g[NQ] g_sc[NQ]
    NCONST = NQC + NKC + KV * HD + NQ + NQ
    consts = inp("consts", [128, NCONST], F32)
    rope_m = inp("rope_m", [128, 128], BF16)
    csq = inp("csq", [128, 2, NQ], BF16)     # [cos_q; sin_q]
    csk = inp("csk", [128, 2, T], BF16)      # [cos_k; sin_k]
    maskq = inp("maskq", [128, NT, NQ], BF16)
    selg = inp("selg", [128, ND, NQ], F32)   # selresT * (1 - g)

    updT = nc.dram_tensor("updT", [128, ND, NQ], F32, kind="ExternalOutput").ap()
    dbg_o = {}
    if dbg:
        for nm, shp, dt_ in [("d_normT", [128, ND, T], F8),
                             ("d_kT", [128, NKC, T], BF16),
                             ("d_vplus", [128, NT, KV, HD + 2], BF16),
                             ("d_qrT", [128, NQC, NQ], BF16),
                             ("d_ctxT", [128, NQC, NQ], F8),
                             ("d_hTt", [128, ND, NQ], F32),
                             ("d_n2T", [128, ND, NQ], F8),
                             ("d_actT", [128, NFC, NQ], F8)]:
            dbg_o[nm] = nc.dram_tensor(nm, shp, dt_, kind="ExternalOutput").ap()

    with tile.TileContext(nc, pool_alloc_mode="queue") as tc:
        with tc.tile_pool(name="ps", bufs=8, space="PSUM") as ps, \
             tc.tile_pool(name="persist", bufs=1) as pp, \
             tc.tile_pool(name="rows", bufs=2) as rowp, \
             tc.tile_pool(name="raw", bufs=2) as rawp:

            # ---- persistent tiles ------------------------------------------------
            selT = pp.tile([128, ND, NQ], F32, name="selT")
            nselT = pp.tile([128, ND, NQ], F8, name="nselT")
            qrT = pp.tile([128, NQC, NQ], BF16, name="qrT")
            normT8 = pp.tile([128, ND, T], F8, name="normT8")
            kT = pp.tile([128, NKC, T], BF16, name="kT")
            vplus = pp.tile([128, NT, KV, HD + 2], BF16, name="vplus")
            ctxT = pp.tile([128, NQC, NQ], F8, name="ctxT")
            hTt = pp.tile([128, ND, NQ], F32, name="hTt")
            n2T = pp.tile([128, ND, NQ], F8, name="n2T")
            actT = pp.tile([128, NFC, NQ], F8, name="actT")

            w_q = pp.tile([128, ND, H * HD], F8, name="w_q")
            w_k = pp.tile([128, ND, KV * HD], F8, name="w_k")
            w_v = pp.tile([128, ND, KV * HD], F8, name="w_v")
            w_o = pp.tile([128, NQC, D], F8, name="w_o")
            selgT = pp.tile([128, ND, NQ], F32, name="selgT")

            c_const = pp.tile([128, NCONST], F32, name="c_const")
            c_qb = c_const[:, 0:NQC]
            c_kb = c_const[:, NQC:NQC + NKC]
            c_vb = c_const[:, NQC + NKC:NQC + NKC + KV * HD]
            OG = NQC + NKC + KV * HD
            c_g = c_const[:, OG:OG + NQ]
            c_gsc = c_const[:, OG + NQ:OG + 2 * NQ]
            c_rm = pp.tile([128, 128], BF16, name="c_rm")
            c_csq = pp.tile([128, 2, NQ], BF16, name="c_csq")
            c_csk = pp.tile([128, 2, T], BF16, name="c_csk")
            c_mask = pp.tile([128, NT, NQ], BF16, name="c_mask")

            ones_t = pp.tile([128, 1], BF16, name="ones_t")
            nc.vector.memset(ones_t, 1.0)
            eps_t = pp.tile([1, 1], F32, name="eps_t")
            nc.vector.memset(eps_t, EPS)
            ones_all = pp.tile([128, 128], F32, name="ones_all")
            nc.vector.memset(ones_all, 1.0)
            nc.vector.memset(vplus[:, :, :, 0:1], 1.0)
            nc.vector.memset(vplus[:, :, :, HD + 1:HD + 2], 1.0)

            # ---- front DMAs (order per queue = emission order) -------------------
            nc.sync.dma_start(out=selT, in_=selres)
            nc.scalar.dma_start(out=c_const, in_=consts)
            nc.scalar.dma_start(out=c_csq, in_=csq)
            nc.scalar.dma_start(out=c_rm, in_=rope_m)
            nc.gpsimd.dma_start(out=w_q, in_=qw8)
            nc.gpsimd.dma_start(out=w_k, in_=kw8)
            nc.gpsimd.dma_start(out=w_v, in_=vw8)

            raw_t = [None] * 4
            raw_t[0] = rawp.tile([128, ND, 512], BF16, name="raw0", tag="raw")
            nc.sync.dma_start(out=raw_t[0], in_=hidc[0])
            raw_t[1] = rawp.tile([128, ND, 512], BF16, name="raw1", tag="raw")
            nc.scalar.dma_start(out=raw_t[1], in_=hidc[1])
            nc.scalar.dma_start(out=c_csk, in_=csk)
            nc.gpsimd.dma_start(out=c_mask, in_=maskq)
            nc.gpsimd.dma_start(out=w_o, in_=ow8)
            nc.gpsimd.dma_start(out=selgT, in_=selg)

            # ======================================================================
            # Phase 3 (emitted after emit_norm(0) below): selected-row rmsnorm +
            # Q proj + rope. PE fills the first chunk's norm-chain latency.
            # ======================================================================
            p3_cm = tc.tile_pool(name="ph3", bufs=3)
            p3 = p3_cm.__enter__()

            def _ph3():
                sq3 = p3.tile([128, ND, NQ], BF16, name="sq3", bufs=1)
                nc.vector.tensor_mul(out=sq3, in0=selT, in1=selT)
                ssq = ps.tile([128, 512], F32, name="ssq", tag="ps")
                for dt in range(ND):
                    nc.tensor.matmul(ssq[0:1, 0:NQ], lhsT=ones_t, rhs=sq3[:, dt, :],
                                     start=(dt == 0), stop=(dt == ND - 1))
                # rstd = exp(-0.5*ln(ms+eps)): keeps the ACT engine on the
                # ln+exp table (no swaps, no slow DVE reciprocal)
                lrow3 = rowp.tile([1, NQ], F32, name="lrow3", tag="row")
                nc.scalar.activation(out=lrow3, in_=ssq[0:1, 0:NQ], func=AF.Ln,
                                     bias=eps_t[0:1, 0:1], scale=1.0 / D)
                rrow3 = rowp.tile([1, NQ], F32, name="rrow3", tag="row")
                nc.scalar.activation(out=rrow3, in_=lrow3, func=AF.Exp,
                                     scale=-0.5)
                rbc3 = ps.tile([128, 512], F32, name="rbc3", tag="ps")
                nc.tensor.matmul(rbc3[:, 0:NQ], lhsT=ones_all[0:1, :], rhs=rrow3,
                                 start=True, stop=True)
                rbc3_sb = p3.tile([128, NQ], F32, name="rbc3_sb", bufs=1)
                nc.vector.tensor_copy(out=rbc3_sb, in_=rbc3[:, 0:NQ])
                nc.vector.tensor_mul(out=nselT, in0=selT,
                                     in1=_bcast_mid(rbc3_sb, ND))

                for qc in range(NQC):
                    qps = ps.tile([128, 512], F32, name="qps", tag="ps")
                    for d2 in range(ND // 2):
                        nc.tensor.matmul(
                            qps[:, 0:NQ],
                            lhsT=w_q[:, 2 * d2:2 * d2 + 2, qc * 128:(qc + 1) * 128],
                            rhs=nselT[:, 2 * d2:2 * d2 + 2, :],
                            start=(d2 == 0), stop=(d2 == ND // 2 - 1),
                            perf_mode=DR)
                    qraw = p3.tile([128, NQ], BF16, name="qraw")
                    nc.vector.tensor_scalar(
                        out=qraw, in0=qps[:, 0:NQ], scalar1=1.0 / SQ,
                        scalar2=c_qb[:, qc:qc + 1], op0=OP.mult, op1=OP.add)
                    rotq = ps.tile([128, 512], F32, name="rotq", tag="ps")
                    nc.tensor.matmul(rotq[:, 0:NQ], lhsT=c_rm, rhs=qraw,
                                     start=True, stop=True)
                    dst = qrT[:, qc, :]
                    tmpq = p3.tile([128, NQ], BF16, name="tmpq")
                    nc.vector.tensor_mul(out=tmpq, in0=rotq[:, 0:NQ],
                                         in1=c_csq[:, 1, :])
                    nc.vector.tensor_mul(out=dst, in0=qraw, in1=c_csq[:, 0, :])
                    nc.vector.tensor_add(out=dst, in0=dst, in1=tmpq)

            # ======================================================================
            # Phases 1+2 fused: rmsnorm runs one 512-token chunk AHEAD of that
            # chunk's K/V projections, so the serial norm chain (ssp -> rsqrt ->
            # broadcast -> scale) hides under the previous chunk's PE work.
            # ======================================================================
            p1_cm = tc.tile_pool(name="ph1", bufs=3)
            p1 = p1_cm.__enter__()
            p2_cm = tc.tile_pool(name="ph2", bufs=3)
            p2 = p2_cm.__enter__()

            def emit_norm(ch):
                if ch + 2 < 4:
                    raw_t[ch + 2] = rawp.tile([128, ND, 512], BF16,
                                              name=f"raw{ch + 2}", tag="raw")
                    eng = nc.sync if (ch + 2) % 2 == 0 else nc.scalar
                    eng.dma_start(out=raw_t[ch + 2], in_=hidc[ch + 2])
                raw = raw_t[ch]
                cs = slice(ch * 512, (ch + 1) * 512)
                sq = p1.tile([128, ND, 512], BF16, name="sq", bufs=2)
                nc.vector.tensor_mul(out=sq, in0=raw, in1=raw)
                ssp = ps.tile([128, 512], F32, name="ssp", tag="ps")
                for dt in range(ND):
                    nc.tensor.matmul(ssp[0:1, :], lhsT=ones_t, rhs=sq[:, dt, :],
                                     start=(dt == 0), stop=(dt == ND - 1))
                lrow = rowp.tile([1, 512], F32, name="lrow", tag="row")
                nc.scalar.activation(out=lrow, in_=ssp[0:1, :], func=AF.Ln,
                                     bias=eps_t[0:1, 0:1], scale=1.0 / D)
                rrow = rowp.tile([1, 512], F32, name="rrow", tag="row")
                nc.scalar.activation(out=rrow, in_=lrow, func=AF.Exp,
                                     scale=-0.5)
                rbc = ps.tile([128, 512], F32, name="rbc", tag="ps")
                nc.tensor.matmul(rbc, lhsT=ones_all[0:1, :], rhs=rrow,
                                 start=True, stop=True)
                rbc_sb = p1.tile([128, 512], BF16, name="rbc_sb", bufs=2)
                nc.vector.tensor_copy(out=rbc_sb, in_=rbc)
                nc.vector.tensor_mul(out=normT8[:, :, cs], in0=raw,
                                     in1=_bcast_mid(rbc_sb, ND))

            def emit_kv(ch):
                    cs = slice(ch * 512, (ch + 1) * 512)
                    # K for this chunk
                    for kc in range(NKC):
                        kps = ps.tile([128, 512], F32, name="kps", tag="ps")
                        for d2 in range(ND // 2):
                            nc.tensor.matmul(
                                kps,
                                lhsT=w_k[:, 2 * d2:2 * d2 + 2, kc * 128:(kc + 1) * 128],
                                rhs=normT8[:, 2 * d2:2 * d2 + 2, cs],
                                start=(d2 == 0), stop=(d2 == ND // 2 - 1),
                                perf_mode=DR)
                        kraw = p2.tile([128, 512], BF16, name="kraw")
                        nc.vector.tensor_scalar(
                            out=kraw, in0=kps, scalar1=1.0 / SK,
                            scalar2=c_kb[:, kc:kc + 1], op0=OP.mult, op1=OP.add)
                        rot = ps.tile([128, 512], F32, name="rot", tag="ps")
                        nc.tensor.matmul(rot, lhsT=c_rm, rhs=kraw,
                                         start=True, stop=True)
                        dst = kT[:, kc, cs]
                        tmp = p2.tile([128, 512], BF16, name="tmp")
                        nc.vector.tensor_mul(out=tmp, in0=rot, in1=c_csk[:, 1, cs])
                        nc.vector.tensor_mul(out=dst, in0=kraw, in1=c_csk[:, 0, cs])
                        nc.vector.tensor_add(out=dst, in0=dst, in1=tmp)

                    # V for this chunk's 4 key tiles
                    for tt in range(ch * 4, ch * 4 + 4):
                        vps = ps.tile([128, 512], F32, name="vps", tag="ps")
                        for d2 in range(ND // 2):
                            nc.tensor.matmul(
                                vps,
                                lhsT=normT8[:, 2 * d2:2 * d2 + 2,
                                            tt * 128:(tt + 1) * 128],
                                rhs=w_v[:, 2 * d2:2 * d2 + 2, :],
                                start=(d2 == 0), stop=(d2 == ND // 2 - 1),
                                perf_mode=DR)
                        nc.vector.scalar_tensor_tensor(
                            out=vplus[:, tt, :, 1:HD + 1],
                            in0=vps.rearrange("p (h d) -> p h d", h=KV),
                            scalar=1.0 / SV,
                            in1=c_vb.rearrange("p (h d) -> p h d", h=KV),
                            op0=OP.mult, op1=OP.add)

            _ph3()
            emit_norm(0)
            emit_norm(1)
            emit_kv(0)
            emit_norm(2)
            emit_kv(1)
            emit_norm(3)
            emit_kv(2)
            emit_kv(3)
            p2_cm.__exit__(None, None, None)
            p1_cm.__exit__(None, None, None)
            p3_cm.__exit__(None, None, None)

            if dbg:
                nc.scalar.dma_start(out=dbg_o["d_normT"], in_=normT8)
                nc.scalar.dma_start(out=dbg_o["d_kT"], in_=kT)
                nc.scalar.dma_start(out=dbg_o["d_vplus"], in_=vplus)
                nc.scalar.dma_start(out=dbg_o["d_qrT"], in_=qrT)

            # ======================================================================
            # Phase 4: attention. 2 groups x 2 kv-chunks; per key tile: scores
            # (K=64 row-group pairs), exp, partial causal mask, ctx accumulate.
            # ctx psum pair tile packs 2 heads along free dim; all at parts 0:65.
            # Software-pipelined at two levels: scores of tile t+1 are emitted
            # before ctx of tile t (PE never waits on the exp/mask chain), and
            # group kc+1's t-loop is emitted before group kc's eviction.
            # ======================================================================
            with tc.tile_pool(name="ph4", bufs=1) as p4:
                live = [t_ for t_ in range(NT) if qlo[t_] < NQ]
                last_tt = max(live)
                cps_all = {}

                def attn_tloop(kc):
                    for ab in range(2):
                        cps_all[(kc, ab)] = ps.tile([128, 512], F32,
                                                    name=f"cps{kc}{ab}", tag="ps")

                    pts = {}

                    def emit_scores(tt):
                        lo = qlo[tt]
                        for half in range(2):
                            # one bank holds the same row-group half of both
                            # q-tiles (A, B) -> same-bank PE writes stay
                            # serial; cross-bank halves run concurrently.
                            hs_ = slice(half * 64, (half + 1) * 64)
                            sp = ps.tile([128, 512], F32, name="sp", tag="ps")
                            for ab in range(2):
                                nc.tensor.matmul(
                                    sp[:, ab * NQ + lo:ab * NQ + NQ],
                                    lhsT=kT[hs_, kc, tt * 128:(tt + 1) * 128],
                                    rhs=qrT[hs_, 2 * kc + ab, lo:NQ],
                                    start=(ab == 0), stop=(ab == 1))
                            pt = p4.tile([128, 2, NQ], BF16, name="pt", bufs=8)
                            nc.scalar.activation(
                                out=pt[:, :, lo:NQ],
                                in_=sp.rearrange("p (h q) -> p h q", h=2)[:, :, lo:NQ],
                                func=AF.Exp)
                            hi = qhi[tt]
                            if hi > lo:
                                nc.vector.tensor_mul(
                                    out=pt[:, :, lo:hi],
                                    in0=pt[:, :, lo:hi],
                                    in1=_bcast_mid(c_mask[:, tt, lo:hi], 2))
                            pts[(tt, half)] = (sp, pt)

                    def emit_ctx(tt):
                        lo = qlo[tt]
                        for half in range(2):
                            sp, pt = pts.pop((tt, half))
                            kvh = 2 * kc + half
                            for ab in range(2):
                                cp = cps_all[(kc, ab)]
                                # start/stop once per PSUM BANK (zero region)
                                nc.tensor.matmul(
                                    cp[0:HD + 1, half * NQ + lo:half * NQ + NQ],
                                    lhsT=vplus[:, tt, kvh, 1:HD + 2],
                                    rhs=pt[:, ab, lo:NQ],
                                    start=(tt == live[0] and half == 0),
                                    stop=(tt == last_tt and half == 1))

                    for i, tt in enumerate(live):
                        emit_scores(tt)
                        if i >= 1:
                            emit_ctx(live[i - 1])
                    emit_ctx(live[-1])

                def attn_evict(kc):
                    # scale by 1/rowsum; odd halves relocated to partitions
                    # 64:128 via SBUF->SBUF DMA (DVE can't cross partitions)
                    for ab in range(2):
                        cp = cps_all[(kc, ab)]
                        rl = p4.tile([128, 512], F32, name="rl", bufs=2)
                        nc.scalar.activation(out=rl[64:65, :], in_=cp[HD:HD + 1, :],
                                             func=AF.Ln)
                        rr = p4.tile([128, 512], F32, name="rr", bufs=2)
                        nc.scalar.activation(out=rr[64:65, :], in_=rl[64:65, :],
                                             func=AF.Exp, scale=-1.0)
                        rb = ps.tile([128, 512], F32, name="rb", tag="ps")
                        nc.tensor.matmul(rb[0:64, :],
                                         lhsT=ones_all[64:65, 0:64],
                                         rhs=rr[64:65, :],
                                         start=True, stop=True)
                        rb_sb = p4.tile([64, 512], F32, name="rb_sb", bufs=2)
                        nc.vector.tensor_copy(out=rb_sb, in_=rb[0:64, :])
                        nc.vector.tensor_mul(
                            out=ctxT[0:64, 2 * kc + ab, :],
                            in0=cp[0:HD, 0:NQ], in1=rb_sb[:, 0:NQ])
                        stage = p4.tile([64, NQ], F8, name="stage", bufs=2)
                        nc.vector.tensor_mul(
                            out=stage, in0=cp[0:HD, NQ:2 * NQ],
                            in1=rb_sb[:, NQ:2 * NQ])
                        nc.sync.dma_start(
                            out=ctxT[64:128, 2 * kc + ab, :], in_=stage)

                # software-pipelined: next group's t-loop is emitted before the
                # previous group's eviction so PE never waits on evictions.
                attn_tloop(0)
                attn_tloop(1)
                attn_evict(0)
                attn_tloop(2)
                attn_evict(1)
                attn_tloop(3)
                attn_evict(2)
                attn_evict(3)

            # ======================================================================
            # Phase 5: o-proj + residual -> hTt (fp32)
            # ======================================================================
            with tc.tile_pool(name="ph5", bufs=2) as p5:
                for dc in range(ND):
                    ops_ = ps.tile([128, 512], F32, name="ops_", tag="ps")
                    for h2 in range(NQC // 2):
                        nc.tensor.matmul(
                            ops_[:, 0:NQ],
                            lhsT=w_o[:, 2 * h2:2 * h2 + 2, dc * 128:(dc + 1) * 128],
                            rhs=ctxT[:, 2 * h2:2 * h2 + 2, :],
                            start=(h2 == 0), stop=(h2 == NQC // 2 - 1),
                            perf_mode=DR)
                    nc.vector.scalar_tensor_tensor(
                        out=hTt[:, dc, :], in0=ops_[:, 0:NQ], scalar=1.0 / SO,
                        in1=selT[:, dc, :], op0=OP.mult, op1=OP.add)

            if dbg:
                nc.scalar.dma_start(out=dbg_o["d_ctxT"], in_=ctxT)
                nc.scalar.dma_start(out=dbg_o["d_hTt"], in_=hTt)

            # ======================================================================
            # Phase 6: rmsnorm2 -> n2T (fp8); then hTt := g*hTt + selg (the
            # gated-residual part that phase 8 adds to the scaled mps).
            # ======================================================================
            with tc.tile_pool(name="ph6", bufs=3) as p6:
                sq6 = p6.tile([128, ND, NQ], BF16, name="sq6", bufs=1)
                nc.vector.tensor_mul(out=sq6, in0=hTt, in1=hTt)
                ss2 = ps.tile([128, 512], F32, name="ss2", tag="ps")
                for dt in range(ND):
                    nc.tensor.matmul(ss2[0:1, 0:NQ], lhsT=ones_t, rhs=sq6[:, dt, :],
                                     start=(dt == 0), stop=(dt == ND - 1))
                lrow6 = rowp.tile([1, NQ], F32, name="lrow6", tag="row")
                nc.scalar.activation(out=lrow6, in_=ss2[0:1, 0:NQ], func=AF.Ln,
                                     bias=eps_t[0:1, 0:1], scale=1.0 / D)
                rrow6 = rowp.tile([1, NQ], F32, name="rrow6", tag="row")
                nc.scalar.activation(out=rrow6, in_=lrow6, func=AF.Exp,
                                     scale=-0.5)
                rbc6 = ps.tile([128, 512], F32, name="rbc6", tag="ps")
                nc.tensor.matmul(rbc6[:, 0:NQ], lhsT=ones_all[0:1, :], rhs=rrow6,
                                 start=True, stop=True)
                rbc6_sb = p6.tile([128, NQ], F32, name="rbc6_sb", bufs=1)
                nc.vector.tensor_copy(out=rbc6_sb, in_=rbc6[:, 0:NQ])
                nc.vector.tensor_mul(out=n2T, in0=hTt,
                                     in1=_bcast_mid(rbc6_sb, ND))
                nc.vector.tensor_mul(out=hTt, in0=hTt, in1=_bcast_mid(c_g, ND))
                nc.vector.tensor_add(out=hTt, in0=hTt, in1=selgT)

            if dbg:
                nc.scalar.dma_start(out=dbg_o["d_n2T"], in_=n2T)

            # ======================================================================
            # Phase 7: MLP gate/up (fp8 DoubleRow) -> actT (fp8)
            # ======================================================================
            with tc.tile_pool(name="ph7w", bufs=3) as p7w, \
                 tc.tile_pool(name="ph7", bufs=3) as p7:
                NGRP = 4
                for g0 in range(0, NFC, NGRP):
                    wg_t = p7w.tile([128, NGRP, ND, 128], F8, name="wg_t")
                    nc.sync.dma_start(out=wg_t, in_=gw8[:, g0:g0 + NGRP])
                    wu_t = p7w.tile([128, NGRP, ND, 128], F8, name="wu_t")
                    nc.scalar.dma_start(out=wu_t, in_=uw8[:, g0:g0 + NGRP])
                    for j in range(NGRP):
                        fc = g0 + j
                        gps = ps.tile([128, 512], F32, name="gps", tag="ps")
                        ups = ps.tile([128, 512], F32, name="ups", tag="ps")
                        for d2 in range(ND // 2):
                            nc.tensor.matmul(
                                gps[:, 0:NQ], lhsT=wg_t[:, j, 2 * d2:2 * d2 + 2, :],
                                rhs=n2T[:, 2 * d2:2 * d2 + 2, :],
                                start=(d2 == 0), stop=(d2 == ND // 2 - 1),
                                perf_mode=DR)
                        for d2 in range(ND // 2):
                            nc.tensor.matmul(
                                ups[:, 0:NQ], lhsT=wu_t[:, j, 2 * d2:2 * d2 + 2, :],
                                rhs=n2T[:, 2 * d2:2 * d2 + 2, :],
                                start=(d2 == 0), stop=(d2 == ND // 2 - 1),
                                perf_mode=DR)
                        sg = p7.tile([128, NQ], BF16, name="sg")
                        nc.scalar.activation(out=sg, in_=gps[:, 0:NQ], func=AF.Silu,
                                             scale=1.0 / SG)
                        nc.vector.tensor_mul(out=actT[:, fc, :], in0=ups[:, 0:NQ],
                                             in1=sg)

            if dbg:
                nc.scalar.dma_start(out=dbg_o["d_actT"], in_=actT)

            # ======================================================================
            # Phase 8: down-proj (fp8 DoubleRow) + gated residual update.
            # f1 written into selT (free by now); one output DMA at the end.
            # ======================================================================
            with tc.tile_pool(name="ph8w", bufs=2) as p8w:
                for dc in range(ND):
                    wd_t = p8w.tile([128, NFC, 128], F8, name="wd_t")
                    (nc.sync if dc % 2 == 0 else nc.gpsimd).dma_start(
                        out=wd_t, in_=dw8[:, dc])
                    mps = ps.tile([128, 512], F32, name="mps", tag="ps")
                    for f2 in range(NFC // 2):
                        nc.tensor.matmul(
                            mps[:, 0:NQ], lhsT=wd_t[:, 2 * f2:2 * f2 + 2, :],
                            rhs=actT[:, 2 * f2:2 * f2 + 2, :],
                            start=(f2 == 0), stop=(f2 == NFC // 2 - 1),
                            perf_mode=DR)
                    # updated = (selg + g*h) + g*mlp ; mps = SMLP*mlp
                    dst = selT[:, dc, :]
                    nc.vector.tensor_mul(out=dst, in0=mps[:, 0:NQ], in1=c_gsc)
                    nc.vector.tensor_add(out=dst, in0=dst, in1=hTt[:, dc, :])
                nc.gpsimd.dma_start(out=updT, in_=selT)

    _split_excess_waits(nc)
    return nc


# ---------------------------------------------------------------------------
# host side
# ---------------------------------------------------------------------------

def _bf16(x):
    return np.asarray(x, dtype=np.float32).astype(ml_dtypes.bfloat16)


def _f8(x):
    return np.asarray(x, dtype=np.float32).astype(ml_dtypes.float8_e4m3)


def _rope_matrix():
    """R[k, p] = sign(p) * 1[k == swap(p)]; (R.T @ x)[p] = sign(p)*x[swap(p)].

    rot(x)[p%64 < 32] = -x[p+32], else +x[p-32]  (two stacked 64-dim heads).
    """
    R = np.zeros((128, 128), np.float32)
    for p in range(128):
        base = (p // 64) * 64
        off = p % 64
        if off < 32:
            R[base + off + 32, p] = -1.0
        else:
            R[base + off - 32, p] = 1.0
    return R


def _install_ntff_hook():
    """Shim antenv.axon_hooks (absent in this image) so trace=True works."""
    import types
    try:
        import antenv.axon_hooks  # noqa: F401
        return
    except ImportError:
        pass
    try:
        from trn_agent_boot.trn_boot import _ntff_profile_via_ctypes
        hook = _ntff_profile_via_ctypes("/opt/axon/libaxon_pjrt.so")
    except Exception:
        hook = None
    mod = types.ModuleType("antenv.axon_hooks")
    mod._hook = hook
    mod.set_axon_ntff_profile_hook = lambda h: setattr(mod, "_hook", h)
    mod.get_axon_ntff_profile_hook = lambda: mod._hook
    sys.modules["antenv.axon_hooks"] = mod


def kernel(hidden_states, token_indices, batch_indices, gating_scores, cos, sin,
           ln1_w, ln2_w, q_w, q_b, k_w, k_b, v_w, v_b, o_w, gate_w, up_w, down_w,
           _profile=False, _dbg=False):
    hidden_states = np.asarray(hidden_states, dtype=np.float32)
    token_indices = np.asarray(token_indices).astype(np.int64)
    gating_scores = np.asarray(gating_scores, dtype=np.float32)
    cos = np.asarray(cos, dtype=np.float32)
    sin = np.asarray(sin, dtype=np.float32)
    ln1_w = np.asarray(ln1_w, dtype=np.float32)
    ln2_w = np.asarray(ln2_w, dtype=np.float32)

    topk = token_indices.reshape(B, KSEL)
    gsc = gating_scores.reshape(B, KSEL)

    core_pos = []
    for c in range(NCORES):
        b = c // 2
        core_pos.append(np.asarray(topk[b, c % 2::2], dtype=np.int64))

    qlo = [min(int(np.searchsorted(core_pos[c], tt * 128)) for c in range(NCORES))
           for tt in range(NT)]
    qhi = [max(int(np.searchsorted(core_pos[c], tt * 128 + 126, side="right"))
               for c in range(NCORES))
           for tt in range(NT)]

    nc = build_program(qlo, qhi, dbg=_dbg)

    # ---- weights (shared across cores) ----
    q_w_eff = (np.asarray(q_w, np.float32) * ln1_w[None, :]) * (SQ / 8.0)
    k_w_eff = np.asarray(k_w, np.float32) * ln1_w[None, :] * SK
    v_w_eff = np.asarray(v_w, np.float32) * ln1_w[None, :] * SV
    g_w_eff = np.asarray(gate_w, np.float32) * ln2_w[None, :] * SG
    u_w_eff = np.asarray(up_w, np.float32) * ln2_w[None, :] * SU
    q_b_eff = (np.asarray(q_b, np.float32) / 8.0)[HEAD_PERM]

    # layouts: [128, ND, out] with element [p, dt, m] = W_eff[m, dt*128+p]
    qw8 = _f8(np.ascontiguousarray(
        q_w_eff.T[:, HEAD_PERM].reshape(ND, 128, H * HD).transpose(1, 0, 2)))
    kw8 = _f8(np.ascontiguousarray(
        k_w_eff.T.reshape(ND, 128, KV * HD).transpose(1, 0, 2)))
    vw8 = _f8(np.ascontiguousarray(
        v_w_eff.T.reshape(ND, 128, KV * HD).transpose(1, 0, 2)))
    ow8 = _f8(np.ascontiguousarray(
        (np.asarray(o_w, np.float32) * SO).T[HEAD_PERM, :]
        .reshape(NQC, 128, D).transpose(1, 0, 2)))
    gw8 = _f8(np.ascontiguousarray(
        g_w_eff.reshape(NFC, 128, ND, 128).transpose(3, 0, 2, 1)))
    uw8 = _f8(np.ascontiguousarray(
        u_w_eff.reshape(NFC, 128, ND, 128).transpose(3, 0, 2, 1)))
    dw8 = _f8(np.ascontiguousarray(
        (np.asarray(down_w, np.float32) * SD)
        .reshape(ND, 128, NFC, 128).transpose(3, 0, 2, 1)))

    qb_a = np.ascontiguousarray(q_b_eff.reshape(NQC, 128).T).astype(np.float32)
    kb_a = np.ascontiguousarray(np.asarray(k_b, np.float32).reshape(NKC, 128).T)
    vb_a = np.broadcast_to(np.asarray(v_b, np.float32)[None, :], (128, KV * HD))

    shared = dict(qw8=qw8, kw8=kw8, vw8=vw8, ow8=ow8, gw8=gw8, uw8=uw8, dw8=dw8,
                  rope_m=_bf16(_rope_matrix()))

    def stack2(a, b):       # two [n, 64] -> [128, 2, n]
        out = np.empty((128, 2, a.shape[0]), np.float32)
        aT = a.T; bT = b.T
        out[:64, 0] = aT; out[64:, 0] = aT
        out[:64, 1] = bT; out[64:, 1] = bT
        return out

    in_maps = []
    for c in range(NCORES):
        b = c // 2
        pos = core_pos[c]
        g_c = gsc[b, c % 2::2].astype(np.float32)
        consts = np.empty((128, NQC + NKC + KV * HD + NQ + NQ), np.float32)
        consts[:, 0:NQC] = qb_a
        consts[:, NQC:NQC + NKC] = kb_a
        consts[:, NQC + NKC:NQC + NKC + KV * HD] = vb_a
        OG = NQC + NKC + KV * HD
        consts[:, OG:OG + NQ] = g_c[None, :]
        consts[:, OG + NQ:OG + 2 * NQ] = (g_c / SMLP)[None, :]

        # causal masks per key tile: maskq[p, tt, q] = 1 if pos[q] >= tt*128+p
        key_abs = (np.arange(NT)[None, :, None] * 128
                   + np.arange(128)[:, None, None])          # [128, NT, 1]
        maskq = (pos[None, None, :] >= key_abs)              # [128, NT, NQ]

        hb = hidden_states[b]                                # [T, D]
        selres = np.ascontiguousarray(
            hb[pos].T.reshape(ND, 128, NQ).transpose(1, 0, 2)).astype(np.float32)
        selg = np.ascontiguousarray(
            (hb[pos] * (1.0 - g_c)[:, None]).T
            .reshape(ND, 128, NQ).transpose(1, 0, 2)).astype(np.float32)
        hidc = np.ascontiguousarray(
            _bf16(hb.T).reshape(ND, 128, 4, 512).transpose(2, 1, 0, 3))

        im = dict(shared)
        im.update(
            selres=selres,
            hidc=hidc,
            consts=consts,
            csq=_bf16(stack2(cos[b][pos], sin[b][pos])),
            csk=_bf16(stack2(cos[b], sin[b])),
            maskq=_bf16(maskq.astype(np.float32)),
            selg=selg,
        )
        in_maps.append(im)

    if _profile:
        _install_ntff_hook()
    res = run_bass_kernel_spmd(nc, in_maps, core_ids=list(range(NCORES)),
                               trace=_profile)

    out = hidden_states.copy()
    for c in range(NCORES):
        b = c // 2
        upd = res.results[c]["updT"].transpose(1, 0, 2).reshape(D, NQ).T
        out[b, core_pos[c], :] = upd
    if _profile or _dbg:
        return out, res
    return out
